# revision 21
# baseline (speedup 1.0000x reference)
"""Blocksparse dilated attention TRN2 kernel (v2).

Sharding: 8 cores = r(=4 dilation offsets) x B(=2 batch). Each core runs one
independent per-offset attention branch on its strided token subset
(x[b, o::r, :]), with that offset's own weights. Host does the strided
gather (+transpose to channel-major) and the final scatter into the
zero-padded (B, S, r*D) output.

Per-core math (L=2048 tokens, D=768, H=12 heads, hd=64, segment=512):
  qkvT = Wqkv @ xoT            (channel-on-partition for q,k; token-major v)
  per (segment, head-pair):
    scoresT = kT-chunks.T x qT   (k on partitions; the two heads of a
          128-channel chunk occupy PE row-groups 0-63/64-127 and stream
          concurrently)
    attnT = exp(scale * scoresT) (ACT; no max-subtract: scores std ~0.3)
    ctxuT = [v_h0 | v_h1].T-packed: two col-tiled M=64 matmuls at array
          col-groups 0-1 / 2-3, streaming each head's attnT concurrently
          -> full 128-col PE utilization (v1 used M=65 = 51%).
  per head-quad w (denominator):
    den = onesT.T x attnT as four col-tiled M=32 matmuls (replicated ones
          columns) at col positions {0,32,64,96}, accumulated over key
          chunks -> one PSUM bank holds 4 heads' denominators.
    den -> (DVE drain) -> (SWDGE spread over 128 partitions) -> DVE
          reciprocal -> rc_dram  (DVE recip is ~6.5ns/elem/lane, hence
          the spread)
  normalize: rc broadcast per head-chunk via SWDGE partition-step-0 DMA
          (DRAM source required), then DVE mul into bf16 ctx_s.
  outT = Wout @ ctx_s + bout   (bf16 output; host upcasts)

Matmuls in bf16 (fp8 measured 5-25x over the 2e-2 error gate on host).

Emission fully software-pipelines across segments: attention(s)'s c-loop
interleaves filler tasks [proj(s+1), outproj(s-1), normalize muls] so the
PE never idles and the ACT exp stream (1.06us per [128,2,512] tile — the
per-iteration co-bottleneck) is hidden.  wout's DMA is gated behind a tiny
DVE memset so the startup HBM burst only carries first-needed bytes.
"""

import math
import sys
from contextlib import ExitStack

import ml_dtypes
import numpy as np

for _p in ("/opt/trn_rl_repo",):
    if _p not in sys.path:
        sys.path.insert(0, _p)

import concourse.bass as bass
import concourse.mybir as mybir
import concourse.tile as tile
from concourse import bacc
from concourse.bass_utils import run_bass_kernel_spmd

P = 128

# Problem constants (hardcoded per harness contract)
B0, S0, D0 = 2, 8192, 768
R0 = 4
H0, HD0 = 12, 64
SEG0 = 512
NSEG0 = (S0 // R0) // SEG0  # 4
N_CORES = 8

F32 = mybir.dt.float32
BF16 = mybir.dt.bfloat16


def build_nc(D=D0, H=H0, HD=HD0, SEG=SEG0, NSEG=NSEG0, mm_dt=BF16, debug=False):
    """Build the per-core Bass program (same NEFF on all cores)."""
    DC = D // P                # channel chunks of 128 (6)
    L = SEG * NSEG             # tokens per core (2048)
    KC = SEG // P              # key chunks per segment (4)
    HPC = P // HD              # heads per 128-channel chunk (2)
    NW = H // 4                # denominator head-quad windows per segment (3)
    E3 = 3 * D
    scale = 1.0 / math.sqrt(HD)
    assert D == H * HD and SEG % P == 0 and D % P == 0 and KC % 2 == 0

    nc = bacc.Bacc(trn_type="TRN2")
    xoT = nc.dram_tensor("xoT", [D, L], mm_dt, kind="ExternalInput")
    wqkvT = nc.dram_tensor("wqkvT", [D, E3], mm_dt, kind="ExternalInput")
    woutT = nc.dram_tensor("woutT", [D, D], mm_dt, kind="ExternalInput")
    bqkv_pt = nc.dram_tensor("bqkv_pt", [P, 3 * DC], F32, kind="ExternalInput")
    bout_pt = nc.dram_tensor("bout_pt", [P, DC], F32, kind="ExternalInput")
    bv = nc.dram_tensor("bv", [D], F32, kind="ExternalInput")
    outT = nc.dram_tensor("outT", [D, L], mm_dt, kind="ExternalOutput")
    # rc staging: the partition-step-0 broadcast DMA needs a DRAM source
    rc_dram = nc.dram_tensor("rc_dram", [NSEG, H * SEG], mm_dt,
                             kind="ExternalOutput" if debug else "Internal")
    if debug:
        dbg = {
            "qk_dbg": nc.dram_tensor("qk_dbg", [NSEG, P, 2 * (D // P), SEG],
                                     mm_dt, kind="ExternalOutput"),
            "v_dbg": nc.dram_tensor("v_dbg", [NSEG, P, SEG // P, D], mm_dt,
                                    kind="ExternalOutput"),
            "at_dbg": nc.dram_tensor("at_dbg", [NSEG, D // P, P, P // HD,
                                                SEG // P, SEG], mm_dt,
                                     kind="ExternalOutput"),
            "den_dbg": nc.dram_tensor("den_dbg", [NSEG, H // 4, P, SEG], F32,
                                      kind="ExternalOutput"),
            "ctxu_dbg": nc.dram_tensor("ctxu_dbg", [NSEG, P, D // P, SEG],
                                       mm_dt, kind="ExternalOutput"),
            "ctxs_dbg": nc.dram_tensor("ctxs_dbg", [NSEG, P, D // P, SEG],
                                       mm_dt, kind="ExternalOutput"),
        }

    with ExitStack() as ctx:
        tc = ctx.enter_context(tile.TileContext(nc))
        singles = ctx.enter_context(tc.tile_pool(name="singles", bufs=1))
        xo_pool = ctx.enter_context(tc.tile_pool(name="xo", bufs=2))
        qk_pool = ctx.enter_context(tc.tile_pool(name="qk", bufs=2))
        v_pool = ctx.enter_context(tc.tile_pool(name="v", bufs=2))
        at_pool = ctx.enter_context(tc.tile_pool(name="attn", bufs=3))
        ctxu_pool = ctx.enter_context(tc.tile_pool(name="ctxu", bufs=2))
        ctxs_pool = ctx.enter_context(tc.tile_pool(name="ctxs", bufs=2))
        out_pool = ctx.enter_context(tc.tile_pool(name="outp", bufs=4))
        bcast_pool = ctx.enter_context(tc.tile_pool(name="bcast", bufs=4))
        den_pool = ctx.enter_context(tc.tile_pool(name="den", bufs=2))
        pp_proj = ctx.enter_context(tc.tile_pool(name="pp_proj", bufs=2, space="PSUM"))
        pp_scA = ctx.enter_context(tc.tile_pool(name="pp_scA", bufs=1, space="PSUM"))
        pp_scB = ctx.enter_context(tc.tile_pool(name="pp_scB", bufs=1, space="PSUM"))
        pp_cb = ctx.enter_context(tc.tile_pool(name="pp_cb", bufs=2, space="PSUM"))

        # --- persistent biases / ones (tiny, first) ---
        bqkv_sb = singles.tile([P, 3 * DC], F32, tag="bqkv")
        nc.sync.dma_start(out=bqkv_sb, in_=bqkv_pt[:, :])
        bout_sb = singles.tile([P, DC], F32, tag="bout")
        nc.sync.dma_start(out=bout_sb, in_=bout_pt[:, :])
        bv_sb = singles.tile([P, D], F32, tag="bv")
        bv_ap = bv[:]
        bv_bcast = bass.AP(tensor=bv_ap.tensor, offset=bv_ap.offset,
                           ap=[[0, P], *bv_ap.ap])
        nc.gpsimd.dma_start(out=bv_sb, in_=bv_bcast)
        ones_sb = singles.tile([P, 1], mm_dt, tag="ones")
        nc.vector.memset(ones_sb, 1.0)

        # --- segment-0 inputs + q,k weight sections interleaved (these gate
        # the first matmuls; v next; wout deferred via a DVE gate) ---
        w_qkv_sb = singles.tile([P, DC, E3], mm_dt, tag="wqkv")
        xo_tiles = {}
        xo0 = xo_pool.tile([P, DC, SEG], mm_dt, tag="xo", name="xo_s0")
        xo_tiles[0] = xo0
        for dc in range(DC):
            nc.sync.dma_start(out=xo0[:, dc, :], in_=xoT[dc * P:(dc + 1) * P, 0:SEG])
        # q section per-(ec, dc) 128x128 chunks so the first proj group's
        # ~1MB arrives at full bandwidth well before the rest of the weights
        for ec in range(DC):
            for dc in range(DC):
                nc.sync.dma_start(
                    out=w_qkv_sb[:, dc, ec * P:(ec + 1) * P],
                    in_=wqkvT[dc * P:(dc + 1) * P, ec * P:(ec + 1) * P])
        for dc in range(DC):
            nc.sync.dma_start(
                out=w_qkv_sb[:, dc, D:2 * D],
                in_=wqkvT[dc * P:(dc + 1) * P, D:2 * D])
        for dc in range(DC):
            nc.sync.dma_start(
                out=w_qkv_sb[:, dc, 2 * D:3 * D],
                in_=wqkvT[dc * P:(dc + 1) * P, 2 * D:3 * D])
        w_out_sb = singles.tile([P, DC, D], mm_dt, tag="wout")

        def wout_task():
            # memset gate: the wout DMAs carry a WAW dep on this DVE op, so
            # they only start mid-attention(0) instead of burning startup HBM
            nc.vector.memset(w_out_sb[:, :, 0:1], 0.0)
            for dc in range(DC):
                nc.sync.dma_start(out=w_out_sb[:, dc, :],
                                  in_=woutT[dc * P:(dc + 1) * P, :])

        # ---------- projection tasks (qkv for one segment) ----------
        def make_proj_tasks(s):
            """Allocate segment-s tiles; return (state, task list)."""
            st = {}
            if s in xo_tiles:
                xo_s = xo_tiles[s]
            else:
                xo_s = xo_pool.tile([P, DC, SEG], mm_dt, tag="xo", name=f"xo_s{s}")
            qk_s = qk_pool.tile([P, 2 * DC, SEG], mm_dt, tag="qk", name=f"qk_s{s}")
            v_s = v_pool.tile([P, KC, D], mm_dt, tag="v", name=f"v_s{s}")
            st["xo"], st["qk"], st["v"] = xo_s, qk_s, v_s

            def xo_task():
                # memset gate: delays the DMAs to this task's slot so they
                # don't steal startup HBM bandwidth at t=0; gpsimd's queue is
                # shallow so the gate itself clears promptly
                nc.gpsimd.memset(xo_s[:, :, 0:1], 0.0)
                for dc in range(DC):
                    nc.sync.dma_start(
                        out=xo_s[:, dc, :],
                        in_=xoT[dc * P:(dc + 1) * P, s * SEG:(s + 1) * SEG])

            def qk_task(ec):
                ps = pp_proj.tile([P, SEG], F32, tag="proj", name=f"psqk{s}_{ec}")
                for dc in range(DC):
                    nc.tensor.matmul(
                        ps,
                        w_qkv_sb[:, dc, ec * P:(ec + 1) * P],
                        xo_s[:, dc, :],
                        start=(dc == 0), stop=(dc == DC - 1))
                nc.vector.tensor_scalar_add(qk_s[:, ec, :], ps, bqkv_sb[:, ec:ec + 1])

            def v_task(lc, n0, n):
                psv = pp_proj.tile([P, SEG], F32, tag="proj", name=f"psv{s}_{lc}_{n0}")
                for dc in range(DC):
                    nc.tensor.matmul(
                        psv[:, :n],
                        xo_s[:, dc, lc * P:(lc + 1) * P],
                        w_qkv_sb[:, dc, 2 * D + n0: 2 * D + n0 + n],
                        start=(dc == 0), stop=(dc == DC - 1))
                nc.vector.tensor_add(v_s[:, lc, n0:n0 + n], psv[:, :n],
                                     bv_sb[:, n0:n0 + n])

            v_list = [(lc, n0, min(512, D - n0))
                      for lc in range(KC) for n0 in range(0, D, 512)]
            if s == 0:
                # prologue: q groups then k then v, matching weight DMA arrival
                tasks = ([lambda ec=c: qk_task(ec) for c in range(2 * DC)]
                         + [lambda a=a: v_task(*a) for a in v_list])
            else:
                tasks = [xo_task]
                vi = 0
                for c in range(DC):
                    tasks.append(lambda ec=c: qk_task(ec))
                    tasks.append(lambda ec=DC + c: qk_task(ec))
                    if vi < len(v_list):
                        tasks.append(lambda a=v_list[vi]: v_task(*a))
                        vi += 1
                while vi < len(v_list):
                    tasks.append(lambda a=v_list[vi]: v_task(*a))
                    vi += 1
            return st, tasks

        # ---------- denominator chain + normalize ----------
        bcs_map = {}

        def den_chain(s, w, den_ps):
            """den_ps holds 4 heads' denominators (replicated x32 rows).
            Drain -> spread over 128 partitions -> reciprocal -> rc_dram."""
            den_sb = den_pool.tile([P, SEG], F32, tag="densb", name=f"dsb{s}_{w}")
            nc.vector.tensor_copy(den_sb, den_ps)
            if debug:
                nc.sync.dma_start(out=dbg["den_dbg"][s, w], in_=den_sb)
            den_t = den_pool.tile([P, 16], F32, tag="dent", name=f"dt{s}_{w}")
            for j in range(4):
                nc.sync.dma_start(out=den_t[32 * j:32 * (j + 1), :],
                                  in_=den_sb[32 * j:32 * j + 1, :])
            rc_t = den_pool.tile([P, 16], mm_dt, tag="rct", name=f"rt{s}_{w}")
            with nc.allow_low_precision(
                    reason="softmax denominator reciprocal; bf16 scale factor"):
                nc.vector.reciprocal(rc_t, den_t)
            nc.sync.dma_start(
                out=rc_dram[s:s + 1, 4 * w * SEG:(4 * w + 4) * SEG], in_=rc_t)

        def bcast(s, hc):
            bcs = bcast_pool.tile([P, SEG], mm_dt, tag="bcs", name=f"bcs{s}_{hc}")
            rr = rc_dram[s:s + 1, hc * HPC * SEG:(hc + 1) * HPC * SEG]
            rr_b = bass.AP(tensor=rr.tensor, offset=rr.offset,
                           ap=[[SEG, HPC], [0, HD], [1, SEG]])
            nc.gpsimd.dma_start(out=bcs, in_=rr_b)
            bcs_map[(s, hc)] = bcs

        def norm_mul(s, hc, stt):
            nc.vector.tensor_mul(stt["ctx_s"][:, hc, :], stt["ctxu"][:, hc, :],
                                 bcs_map.pop((s, hc)))

        # ---------- output projection ----------
        def outproj_task(s, fc, stt):
            pso = pp_proj.tile([P, SEG], F32, tag="proj", name=f"pso{s}_{fc}")
            for dc in range(DC):
                nc.tensor.matmul(
                    pso,
                    w_out_sb[:, dc, fc * P:(fc + 1) * P],
                    stt["ctx_s"][:, dc, :],
                    start=(dc == 0), stop=(dc == DC - 1))
            ot = out_pool.tile([P, SEG], mm_dt, tag="ot", name=f"ot{s}_{fc}")
            nc.vector.tensor_scalar_add(ot, pso, bout_sb[:, fc:fc + 1])
            nc.sync.dma_start(
                out=outT[fc * P:(fc + 1) * P, s * SEG:(s + 1) * SEG], in_=ot)

        # ---------- attention c-loop ----------
        def attention(s, stt, filler):
            qk_s, v_s = stt["qk"], stt["v"]
            ctxu = ctxu_pool.tile([P, DC, SEG], mm_dt, tag="ctxu", name=f"cu{s}")
            ctx_s = ctxs_pool.tile([P, DC, SEG], mm_dt, tag="ctxs", name=f"cs{s}")
            stt["ctxu"], stt["ctx_s"] = ctxu, ctx_s
            ats = {}
            fi = 0
            npts = 3 * (DC + 1)

            def drain(pt):
                nonlocal fi
                want = min(len(filler), int(len(filler) * pt / npts + 0.5))
                while fi < want:
                    filler[fi]()
                    fi += 1

            def den_window(w):
                den_ps = pp_proj.tile([P, SEG], F32, tag="proj",
                                      name=f"dps{s}_{w}")
                for kc in range(KC):
                    for j in range(4):
                        h = 4 * w + j
                        nc.tensor.matmul(
                            den_ps[32 * j:32 * j + 1, :],
                            ones_sb,
                            ats[h // HPC][:, h % HPC, kc, :],
                            start=(kc == 0), stop=(kc == KC - 1),
                            tile_position=(0, 32 * j))
                den_chain(s, w, den_ps)
                bcast(s, 2 * w)
                bcast(s, 2 * w + 1)

            for c in range(DC + 1):
                # den window for a finished chunk pair, before this c's at2
                # alloc can recycle the buffers it reads
                if c in (3, 5):
                    den_window((c - 3) // 2)
                if c < DC:
                    at2 = at_pool.tile([P, HPC, KC, SEG], mm_dt, tag="attn",
                                       name=f"at{s}_{c}")
                    ats[c] = at2
                    for w in range(KC // 2):
                        for half, pool in ((0, pp_scA), (1, pp_scB)):
                            kc = 2 * w + half
                            sc = pool.tile([P, HPC, SEG], F32, tag=f"sc{half}",
                                           name=f"sc{half}_{s}_{c}_{w}")
                            for i in range(HPC):
                                ho = i * HD
                                nc.tensor.matmul(
                                    sc[:, i, :],
                                    qk_s[ho:ho + HD, DC + c, kc * P:(kc + 1) * P],
                                    qk_s[ho:ho + HD, c, :])
                            nc.scalar.activation(
                                at2[:, :, kc, :], sc,
                                mybir.ActivationFunctionType.Exp,
                                scale=scale)
                drain(3 * c + 1)
                if c == DC:
                    # last den window first: its reciprocal chain is the
                    # epilogue critical path, start it before ctx(DC-1)
                    den_window(2)
                if c > 0:
                    cp = c - 1
                    at2 = ats[cp]
                    cps = pp_cb.tile([P, SEG], F32, tag="cb", name=f"cps{s}_{cp}")
                    for kc in range(KC):
                        for i in range(HPC):
                            h = cp * HPC + i
                            nc.tensor.matmul(
                                cps[i * HD:(i + 1) * HD, :],
                                v_s[:, kc, h * HD:(h + 1) * HD],
                                at2[:, i, kc, :],
                                start=(kc == 0), stop=(kc == KC - 1),
                                tile_position=(0, i * HD))
                    nc.vector.tensor_copy(ctxu[:, cp, :], cps)
                    if debug:
                        nc.sync.dma_start(out=dbg["at_dbg"][s, cp],
                                          in_=at2[:, :, :, :])
                drain(3 * c + 2)
                if c >= 3:
                    ats.pop(c - 3, None)
                drain(3 * c + 3)

        # ---------- main pipeline ----------
        sts = {}
        sts[0], tasks0 = make_proj_tasks(0)
        for t in tasks0:
            t()
        for s in range(NSEG):
            nxt = s + 1
            if nxt < NSEG:
                sts[nxt], proj_tasks = make_proj_tasks(nxt)
            else:
                proj_tasks = []
            filler = []
            if s > 0:
                filler.append(lambda a=(s - 1): norm_mul(a, 4, sts[a]))
                filler.append(lambda a=(s - 1): norm_mul(a, 5, sts[a]))
            # interleave prev-segment outproj (from index 4) with next proj
            op_tasks = ([] if s == 0 else
                        [(lambda a=(s - 1), fc=fc: outproj_task(a, fc, sts[a]))
                         for fc in range(DC)])
            merged = []
            pi = oi = 0
            for k in range(len(proj_tasks) + len(op_tasks)):
                take_op = (oi < len(op_tasks)
                           and (k >= 4 and (k - 4) % 4 == 3 or pi >= len(proj_tasks)))
                if take_op:
                    merged.append(op_tasks[oi])
                    oi += 1
                else:
                    merged.append(proj_tasks[pi])
                    pi += 1
            filler += merged
            if s == 0:
                filler.insert(2, wout_task)
            # this segment's early normalize muls (rc ready mid-loop)
            filler += [(lambda hc=hc: norm_mul(s, hc, sts[s])) for hc in range(4)]
            attention(s, sts[s], filler)
            if debug:
                nc.sync.dma_start(out=dbg["qk_dbg"][s], in_=sts[s]["qk"])
                nc.sync.dma_start(out=dbg["v_dbg"][s], in_=sts[s]["v"])
                nc.sync.dma_start(out=dbg["ctxu_dbg"][s], in_=sts[s]["ctxu"])
                if s > 0:
                    nc.sync.dma_start(out=dbg["ctxs_dbg"][s - 1],
                                      in_=sts[s - 1]["ctx_s"])
            if s > 0:
                sts.pop(s - 1)

        # ---------- epilogue: last segment normalize + outproj ----------
        SL = NSEG - 1
        stl = sts[SL]
        norm_mul(SL, 4, stl)
        norm_mul(SL, 5, stl)
        for half in (0, 1):
            fcs = [3 * half + k for k in range(3)]
            psos = [pp_proj.tile([P, SEG], F32, tag="proj", name=f"ep{half}_0"),
                    pp_proj.tile([P, SEG], F32, tag="proj", name=f"ep{half}_1"),
                    pp_cb.tile([P, SEG], F32, tag="cb", name=f"ep{half}_2")]
            for dc in range(DC):
                for k, fc in enumerate(fcs):
                    nc.tensor.matmul(
                        psos[k],
                        w_out_sb[:, dc, fc * P:(fc + 1) * P],
                        stl["ctx_s"][:, dc, :],
                        start=(dc == 0), stop=(dc == DC - 1))
            for k, fc in enumerate(fcs):
                ot = out_pool.tile([P, SEG], mm_dt, tag="ot", name=f"eo{half}_{fc}")
                # DVE drain; ACT would pay a 1.3us table load (Exp -> Identity)
                nc.vector.tensor_scalar_add(ot, psos[k], bout_sb[:, fc:fc + 1])
                nc.sync.dma_start(
                    out=outT[fc * P:(fc + 1) * P, SL * SEG:(SL + 1) * SEG], in_=ot)
        if debug:
            nc.sync.dma_start(out=dbg["ctxs_dbg"][SL], in_=stl["ctx_s"])

    nc.compile()
    return nc


def make_in_maps(x, Wqkv, bqkv, Wout, bout):
    """Shard full inputs across 8 cores: core = o*B + b."""
    r, E3, D = Wqkv.shape
    Bb, S, _ = x.shape
    DC = D // P
    in_maps = []
    for c in range(r * Bb):
        o, b = c // Bb, c % Bb
        in_maps.append({
            "xoT": np.ascontiguousarray(x[b, o::r, :].T).astype(ml_dtypes.bfloat16),
            "wqkvT": np.ascontiguousarray(Wqkv[o].T).astype(ml_dtypes.bfloat16),
            "woutT": np.ascontiguousarray(Wout[o].T).astype(ml_dtypes.bfloat16),
            "bqkv_pt": np.ascontiguousarray(bqkv[o].reshape(3 * DC, P).T),
            "bout_pt": np.ascontiguousarray(bout[o].reshape(DC, P).T),
            "bv": np.ascontiguousarray(bqkv[o, 2 * D:3 * D]),
        })
    return in_maps


_NC_CACHE = {}


def get_nc():
    if "nc" not in _NC_CACHE:
        _NC_CACHE["nc"] = build_nc()
    return _NC_CACHE["nc"]


def run(inputs, trace=False, **kwargs):
    """Run the SPMD kernel; returns (full_output, BassKernelResults)."""
    x = np.ascontiguousarray(np.asarray(inputs["x"], dtype=np.float32))
    Wqkv = np.asarray(inputs["Wqkv"], dtype=np.float32)
    bqkv = np.asarray(inputs["bqkv"], dtype=np.float32)
    Wout = np.asarray(inputs["Wout"], dtype=np.float32)
    bout = np.asarray(inputs["bout"], dtype=np.float32)
    r, E3, D = Wqkv.shape
    Bb, S, _ = x.shape

    nc = get_nc()
    in_maps = make_in_maps(x, Wqkv, bqkv, Wout, bout)
    res = run_bass_kernel_spmd(nc, in_maps, core_ids=list(range(len(in_maps))),
                               trace=trace, **kwargs)

    out = np.zeros((Bb, S, r * D), np.float32)
    for c in range(len(in_maps)):
        o, b = c // Bb, c % Bb
        out[b, o::r, o * D:(o + 1) * D] = \
            np.asarray(res.results[c]["outT"]).astype(np.float32).T
    return out, res


def kernel(x, Wqkv, bqkv, Wout, bout, num_heads):
    assert int(num_heads) == H0
    out, _ = run(dict(x=x, Wqkv=Wqkv, bqkv=bqkv, Wout=Wout, bout=bout))
    return out


# revision 28
# speedup vs baseline: 1.0675x; 1.0675x over previous
"""Blocksparse dilated attention TRN2 kernel (v2).

Sharding: 8 cores = r(=4 dilation offsets) x B(=2 batch). Each core runs one
independent per-offset attention branch on its strided token subset
(x[b, o::r, :]), with that offset's own weights. Host does the strided
gather (+transpose to channel-major) and the final scatter into the
zero-padded (B, S, r*D) output.

Per-core math (L=2048 tokens, D=768, H=12 heads, hd=64, segment=512):
  qkvT = Wqkv @ xoT            (channel-on-partition for q,k; token-major v)
  per (segment, head-pair):
    scoresT = kT-chunks.T x qT   (k on partitions; the two heads of a
          128-channel chunk occupy PE row-groups 0-63/64-127 and stream
          concurrently)
    attnT = exp(scale * scoresT) (ACT; no max-subtract: scores std ~0.3)
    ctxuT = [v_h0 | v_h1].T-packed: two col-tiled M=64 matmuls at array
          col-groups 0-1 / 2-3, streaming each head's attnT concurrently
          -> full 128-col PE utilization (v1 used M=65 = 51%).
  per head-quad w (denominator):
    den = onesT.T x attnT as four col-tiled M=32 matmuls (replicated ones
          columns) at col positions {0,32,64,96}, accumulated over key
          chunks -> one PSUM bank holds 4 heads' denominators.
    den -> (DVE drain) -> (SWDGE spread over 128 partitions) -> DVE
          reciprocal -> rc_dram  (DVE recip is ~6.5ns/elem/lane, hence
          the spread)
  normalize: rc broadcast per head-chunk via SWDGE partition-step-0 DMA
          (DRAM source required), then DVE mul into bf16 ctx_s.
  outT = Wout @ ctx_s + bout   (bf16 output; host upcasts)

Matmuls in bf16 (fp8 measured 5-25x over the 2e-2 error gate on host).

Emission fully software-pipelines across segments: attention(s)'s c-loop
interleaves filler tasks [proj(s+1), outproj(s-1), normalize muls] so the
PE never idles and the ACT exp stream (1.06us per [128,2,512] tile — the
per-iteration co-bottleneck) is hidden.  wout's DMA is gated behind a tiny
DVE memset so the startup HBM burst only carries first-needed bytes.
"""

import math
import sys
from contextlib import ExitStack

import ml_dtypes
import numpy as np

for _p in ("/opt/trn_rl_repo",):
    if _p not in sys.path:
        sys.path.insert(0, _p)

import concourse.bass as bass
import concourse.mybir as mybir
import concourse.tile as tile
from concourse import bacc
from concourse.bass_utils import run_bass_kernel_spmd

P = 128

# Problem constants (hardcoded per harness contract)
B0, S0, D0 = 2, 8192, 768
R0 = 4
H0, HD0 = 12, 64
SEG0 = 512
NSEG0 = (S0 // R0) // SEG0  # 4
N_CORES = 8

F32 = mybir.dt.float32
BF16 = mybir.dt.bfloat16


def build_nc(D=D0, H=H0, HD=HD0, SEG=SEG0, NSEG=NSEG0, mm_dt=BF16, debug=False):
    """Build the per-core Bass program (same NEFF on all cores)."""
    DC = D // P                # channel chunks of 128 (6)
    L = SEG * NSEG             # tokens per core (2048)
    KC = SEG // P              # key chunks per segment (4)
    HPC = P // HD              # heads per 128-channel chunk (2)
    NW = H // 4                # denominator head-quad windows per segment (3)
    E3 = 3 * D
    scale = 1.0 / math.sqrt(HD)
    assert D == H * HD and SEG % P == 0 and D % P == 0 and KC % 2 == 0

    nc = bacc.Bacc(trn_type="TRN2")
    xoT = nc.dram_tensor("xoT", [D, L], mm_dt, kind="ExternalInput")
    wqkvT = nc.dram_tensor("wqkvT", [D, E3], mm_dt, kind="ExternalInput")
    woutT = nc.dram_tensor("woutT", [D, D], mm_dt, kind="ExternalInput")
    bqkv_pt = nc.dram_tensor("bqkv_pt", [P, 3 * DC], F32, kind="ExternalInput")
    bout_pt = nc.dram_tensor("bout_pt", [P, DC], F32, kind="ExternalInput")
    bv = nc.dram_tensor("bv", [D], F32, kind="ExternalInput")
    outT = nc.dram_tensor("outT", [D, L], mm_dt, kind="ExternalOutput")
    # rc staging: the partition-step-0 broadcast DMA needs a DRAM source
    rc_dram = nc.dram_tensor("rc_dram", [NSEG, H * SEG], mm_dt,
                             kind="ExternalOutput" if debug else "Internal")
    if debug:
        dbg = {
            "qk_dbg": nc.dram_tensor("qk_dbg", [NSEG, P, 2 * (D // P), SEG],
                                     mm_dt, kind="ExternalOutput"),
            "v_dbg": nc.dram_tensor("v_dbg", [NSEG, P, SEG // P, D], mm_dt,
                                    kind="ExternalOutput"),
            "at_dbg": nc.dram_tensor("at_dbg", [NSEG, D // P, P, P // HD,
                                                SEG // P, SEG], mm_dt,
                                     kind="ExternalOutput"),
            "den_dbg": nc.dram_tensor("den_dbg", [NSEG, H // 4, P, SEG], F32,
                                      kind="ExternalOutput"),
            "ctxu_dbg": nc.dram_tensor("ctxu_dbg", [NSEG, P, D // P, SEG],
                                       mm_dt, kind="ExternalOutput"),
            "ctxs_dbg": nc.dram_tensor("ctxs_dbg", [NSEG, P, D // P, SEG],
                                       mm_dt, kind="ExternalOutput"),
        }

    with ExitStack() as ctx:
        tc = ctx.enter_context(tile.TileContext(nc))
        singles = ctx.enter_context(tc.tile_pool(name="singles", bufs=1))
        xo_pool = ctx.enter_context(tc.tile_pool(name="xo", bufs=2))
        qk_pool = ctx.enter_context(tc.tile_pool(name="qk", bufs=2))
        v_pool = ctx.enter_context(tc.tile_pool(name="v", bufs=2))
        at_pool = ctx.enter_context(tc.tile_pool(name="attn", bufs=3))
        ctxu_pool = ctx.enter_context(tc.tile_pool(name="ctxu", bufs=2))
        ctxs_pool = ctx.enter_context(tc.tile_pool(name="ctxs", bufs=3))
        out_pool = ctx.enter_context(tc.tile_pool(name="outp", bufs=4))
        bcast_pool = ctx.enter_context(tc.tile_pool(name="bcast", bufs=4))
        den_pool = ctx.enter_context(tc.tile_pool(name="den", bufs=2))
        pp_proj = ctx.enter_context(tc.tile_pool(name="pp_proj", bufs=2, space="PSUM"))
        pp_scA = ctx.enter_context(tc.tile_pool(name="pp_scA", bufs=1, space="PSUM"))
        pp_scB = ctx.enter_context(tc.tile_pool(name="pp_scB", bufs=1, space="PSUM"))
        pp_cb = ctx.enter_context(tc.tile_pool(name="pp_cb", bufs=2, space="PSUM"))

        # --- persistent biases / ones (tiny, first) ---
        bqkv_sb = singles.tile([P, 3 * DC], F32, tag="bqkv")
        nc.sync.dma_start(out=bqkv_sb, in_=bqkv_pt[:, :])
        bout_sb = singles.tile([P, DC], F32, tag="bout")
        nc.sync.dma_start(out=bout_sb, in_=bout_pt[:, :])
        bv_sb = singles.tile([P, D], F32, tag="bv")
        bv_ap = bv[:]
        bv_bcast = bass.AP(tensor=bv_ap.tensor, offset=bv_ap.offset,
                           ap=[[0, P], *bv_ap.ap])
        nc.gpsimd.dma_start(out=bv_sb, in_=bv_bcast)
        ones_sb = singles.tile([P, 1], mm_dt, tag="ones")
        nc.vector.memset(ones_sb, 1.0)

        # --- segment-0 inputs + q,k weight sections interleaved (these gate
        # the first matmuls; v next; wout deferred via a DVE gate) ---
        w_qkv_sb = singles.tile([P, DC, E3], mm_dt, tag="wqkv")
        xo_tiles = {}
        xo0 = xo_pool.tile([P, DC, SEG], mm_dt, tag="xo", name="xo_s0")
        xo_tiles[0] = xo0
        for dc in range(DC):
            nc.sync.dma_start(out=xo0[:, dc, :], in_=xoT[dc * P:(dc + 1) * P, 0:SEG])
            for sec in (0, 1):
                nc.sync.dma_start(
                    out=w_qkv_sb[:, dc, sec * D:(sec + 1) * D],
                    in_=wqkvT[dc * P:(dc + 1) * P, sec * D:(sec + 1) * D])
        for dc in range(DC):
            nc.sync.dma_start(
                out=w_qkv_sb[:, dc, 2 * D:3 * D],
                in_=wqkvT[dc * P:(dc + 1) * P, 2 * D:3 * D])
        w_out_sb = singles.tile([P, DC, D], mm_dt, tag="wout")

        def wout_task():
            # memset gate: the wout DMAs carry a WAW dep on this DVE op, so
            # they only start mid-attention(0) instead of burning startup HBM
            nc.vector.memset(w_out_sb[:, :, 0:1], 0.0)
            for dc in range(DC):
                nc.sync.dma_start(out=w_out_sb[:, dc, :],
                                  in_=woutT[dc * P:(dc + 1) * P, :])

        # ---------- projection tasks (qkv for one segment) ----------
        def make_xo_task(s):
            """Allocate segment-s xo tile + its load task (2-segment lead:
            the pool-WAR dep on the old slot's readers naturally delays the
            DMA past startup without stalling anything)."""
            xo_s = xo_pool.tile([P, DC, SEG], mm_dt, tag="xo", name=f"xo_s{s}")
            xo_tiles[s] = xo_s

            def xo_dma():
                for dc in range(DC):
                    nc.sync.dma_start(
                        out=xo_s[:, dc, :],
                        in_=xoT[dc * P:(dc + 1) * P, s * SEG:(s + 1) * SEG])
            return xo_dma

        def make_proj_tasks(s):
            """Allocate segment-s tiles; return (state, task list)."""
            st = {}
            if s in xo_tiles:
                xo_s = xo_tiles[s]
            else:
                xo_s = xo_pool.tile([P, DC, SEG], mm_dt, tag="xo", name=f"xo_s{s}")
                xo_tiles[s] = xo_s
            qk_s = qk_pool.tile([P, 2 * DC, SEG], mm_dt, tag="qk", name=f"qk_s{s}")
            v_s = v_pool.tile([P, KC, D], mm_dt, tag="v", name=f"v_s{s}")
            st["xo"], st["qk"], st["v"] = xo_s, qk_s, v_s

            def xo_task():
                # s=1 only: DVE memset gate defers the DMAs past the startup
                # HBM burst (DVE reaches it after the first proj drains)
                nc.vector.memset(xo_s[:, :, 0:1], 0.0)
                for dc in range(DC):
                    nc.sync.dma_start(
                        out=xo_s[:, dc, :],
                        in_=xoT[dc * P:(dc + 1) * P, s * SEG:(s + 1) * SEG])

            def qk_task(ec):
                ps = pp_proj.tile([P, SEG], F32, tag="proj", name=f"psqk{s}_{ec}")
                for dc in range(DC):
                    nc.tensor.matmul(
                        ps,
                        w_qkv_sb[:, dc, ec * P:(ec + 1) * P],
                        xo_s[:, dc, :],
                        start=(dc == 0), stop=(dc == DC - 1))
                nc.vector.tensor_scalar_add(qk_s[:, ec, :], ps, bqkv_sb[:, ec:ec + 1])

            def v_task(lc, n0, n):
                psv = pp_proj.tile([P, SEG], F32, tag="proj", name=f"psv{s}_{lc}_{n0}")
                for dc in range(DC):
                    nc.tensor.matmul(
                        psv[:, :n],
                        xo_s[:, dc, lc * P:(lc + 1) * P],
                        w_qkv_sb[:, dc, 2 * D + n0: 2 * D + n0 + n],
                        start=(dc == 0), stop=(dc == DC - 1))
                nc.vector.tensor_add(v_s[:, lc, n0:n0 + n], psv[:, :n],
                                     bv_sb[:, n0:n0 + n])

            v_list = [(lc, n0, min(512, D - n0))
                      for lc in range(KC) for n0 in range(0, D, 512)]
            if s == 0:
                # prologue: q groups then k then v, matching weight DMA arrival
                tasks = ([lambda ec=c: qk_task(ec) for c in range(2 * DC)]
                         + [lambda a=a: v_task(*a) for a in v_list])
            else:
                tasks = [xo_task] if s == 1 else []
                vi = 0
                for c in range(DC):
                    tasks.append(lambda ec=c: qk_task(ec))
                    tasks.append(lambda ec=DC + c: qk_task(ec))
                    if vi < len(v_list):
                        tasks.append(lambda a=v_list[vi]: v_task(*a))
                        vi += 1
                while vi < len(v_list):
                    tasks.append(lambda a=v_list[vi]: v_task(*a))
                    vi += 1
            return st, tasks

        # ---------- denominator chain + normalize ----------
        bcs_map = {}

        def den_chain(s, w, den_ps):
            """den_ps holds 4 heads' denominators (replicated x32 rows).
            Drain -> spread over 128 partitions -> reciprocal -> rc_dram."""
            den_sb = den_pool.tile([P, SEG], F32, tag="densb", name=f"dsb{s}_{w}")
            # ACT drain (Copy needs no activation table): keeps the chain off
            # the backlogged DVE queue
            nc.scalar.copy(den_sb, den_ps)
            if debug:
                nc.sync.dma_start(out=dbg["den_dbg"][s, w], in_=den_sb)
            den_t = den_pool.tile([P, 16], F32, tag="dent", name=f"dt{s}_{w}")
            for j in range(4):
                nc.sync.dma_start(out=den_t[32 * j:32 * (j + 1), :],
                                  in_=den_sb[32 * j:32 * j + 1, :])
            rc_t = den_pool.tile([P, 16], mm_dt, tag="rct", name=f"rt{s}_{w}")
            with nc.allow_low_precision(
                    reason="softmax denominator reciprocal; bf16 scale factor"):
                nc.vector.reciprocal(rc_t, den_t)
            nc.sync.dma_start(
                out=rc_dram[s:s + 1, 4 * w * SEG:(4 * w + 4) * SEG], in_=rc_t)

        def bcast(s, hc):
            bcs = bcast_pool.tile([P, SEG], mm_dt, tag="bcs", name=f"bcs{s}_{hc}")
            rr = rc_dram[s:s + 1, hc * HPC * SEG:(hc + 1) * HPC * SEG]
            rr_b = bass.AP(tensor=rr.tensor, offset=rr.offset,
                           ap=[[SEG, HPC], [0, HD], [1, SEG]])
            nc.gpsimd.dma_start(out=bcs, in_=rr_b)
            bcs_map[(s, hc)] = bcs

        def norm_mul(s, hc, stt):
            nc.vector.tensor_mul(stt["ctx_s"][:, hc, :], stt["ctxu"][:, hc, :],
                                 bcs_map.pop((s, hc)))

        # ---------- output projection ----------
        def outproj_task(s, fc, stt):
            pso = pp_proj.tile([P, SEG], F32, tag="proj", name=f"pso{s}_{fc}")
            for dc in range(DC):
                nc.tensor.matmul(
                    pso,
                    w_out_sb[:, dc, fc * P:(fc + 1) * P],
                    stt["ctx_s"][:, dc, :],
                    start=(dc == 0), stop=(dc == DC - 1))
            ot = out_pool.tile([P, SEG], mm_dt, tag="ot", name=f"ot{s}_{fc}")
            nc.vector.tensor_scalar_add(ot, pso, bout_sb[:, fc:fc + 1])
            nc.sync.dma_start(
                out=outT[fc * P:(fc + 1) * P, s * SEG:(s + 1) * SEG], in_=ot)

        # ---------- attention c-loop ----------
        def attention(s, stt, filler):
            qk_s, v_s = stt["qk"], stt["v"]
            ctxu = ctxu_pool.tile([P, DC, SEG], mm_dt, tag="ctxu", name=f"cu{s}")
            ctx_s = ctxs_pool.tile([P, DC, SEG], mm_dt, tag="ctxs", name=f"cs{s}")
            stt["ctxu"], stt["ctx_s"] = ctxu, ctx_s
            ats = {}
            fi = 0
            npts = 3 * (DC + 1)

            def drain(pt):
                nonlocal fi
                want = min(len(filler), int(len(filler) * pt / npts + 0.5))
                while fi < want:
                    filler[fi]()
                    fi += 1

            def den_window(w):
                den_ps = pp_proj.tile([P, SEG], F32, tag="proj",
                                      name=f"dps{s}_{w}")
                for kc in range(KC):
                    for j in range(4):
                        h = 4 * w + j
                        nc.tensor.matmul(
                            den_ps[32 * j:32 * j + 1, :],
                            ones_sb,
                            ats[h // HPC][:, h % HPC, kc, :],
                            start=(kc == 0), stop=(kc == KC - 1),
                            tile_position=(0, 32 * j))
                den_chain(s, w, den_ps)
                bcast(s, 2 * w)
                bcast(s, 2 * w + 1)

            for c in range(DC + 1):
                # den window for a finished chunk pair, before this c's at2
                # alloc can recycle the buffers it reads
                if c in (3, 5):
                    den_window((c - 3) // 2)
                if c < DC:
                    at2 = at_pool.tile([P, HPC, KC, SEG], mm_dt, tag="attn",
                                       name=f"at{s}_{c}")
                    ats[c] = at2
                    for w in range(KC // 2):
                        for half, pool in ((0, pp_scA), (1, pp_scB)):
                            kc = 2 * w + half
                            sc = pool.tile([P, HPC, SEG], F32, tag=f"sc{half}",
                                           name=f"sc{half}_{s}_{c}_{w}")
                            for i in range(HPC):
                                ho = i * HD
                                nc.tensor.matmul(
                                    sc[:, i, :],
                                    qk_s[ho:ho + HD, DC + c, kc * P:(kc + 1) * P],
                                    qk_s[ho:ho + HD, c, :])
                            nc.scalar.activation(
                                at2[:, :, kc, :], sc,
                                mybir.ActivationFunctionType.Exp,
                                scale=scale)
                drain(3 * c + 1)
                if c == DC:
                    # last den window first: its reciprocal chain is the
                    # epilogue critical path, start it before ctx(DC-1)
                    den_window(2)
                if c > 0:
                    cp = c - 1
                    at2 = ats[cp]
                    cps = pp_cb.tile([P, SEG], F32, tag="cb", name=f"cps{s}_{cp}")
                    for kc in range(KC):
                        for i in range(HPC):
                            h = cp * HPC + i
                            nc.tensor.matmul(
                                cps[i * HD:(i + 1) * HD, :],
                                v_s[:, kc, h * HD:(h + 1) * HD],
                                at2[:, i, kc, :],
                                start=(kc == 0), stop=(kc == KC - 1),
                                tile_position=(0, i * HD))
                    nc.vector.tensor_copy(ctxu[:, cp, :], cps)
                    if debug:
                        nc.sync.dma_start(out=dbg["at_dbg"][s, cp],
                                          in_=at2[:, :, :, :])
                drain(3 * c + 2)
                if c >= 3:
                    ats.pop(c - 3, None)
                drain(3 * c + 3)

        # ---------- main pipeline ----------
        sts = {}
        sts[0], tasks0 = make_proj_tasks(0)
        for t in tasks0:
            t()
        for s in range(NSEG):
            nxt = s + 1
            if nxt < NSEG:
                sts[nxt], proj_tasks = make_proj_tasks(nxt)
            else:
                proj_tasks = []
            filler = []
            if s + 2 < NSEG:
                filler.append(make_xo_task(s + 2))
            if s > 0:
                filler.append(lambda a=(s - 1): norm_mul(a, 4, sts[a]))
                filler.append(lambda a=(s - 1): norm_mul(a, 5, sts[a]))
            # prev-segment outproj, rebalanced so the proj-less attention(3)
            # keeps enough PE filler to never idle past the 3.4us HAM window
            if s == 1:
                op_specs = [(0, fc) for fc in range(DC)]
            elif s == 2:
                op_specs = [(1, fc) for fc in range(3)]
            elif s == 3:
                op_specs = ([(1, fc) for fc in range(3, DC)]
                            + [(2, fc) for fc in range(DC)])
            else:
                op_specs = []
            op_tasks = [(lambda a=a, fc=fc: outproj_task(a, fc, sts[a]))
                        for a, fc in op_specs]
            merged = []
            pi = oi = 0
            for k in range(len(proj_tasks) + len(op_tasks)):
                take_op = (oi < len(op_tasks)
                           and (k >= 4 and (k - 4) % 4 == 3 or pi >= len(proj_tasks)))
                if take_op:
                    merged.append(op_tasks[oi])
                    oi += 1
                else:
                    merged.append(proj_tasks[pi])
                    pi += 1
            filler += merged
            if s == 0:
                filler.insert(2, wout_task)
            # this segment's early normalize muls (rc ready mid-loop)
            filler += [(lambda hc=hc: norm_mul(s, hc, sts[s])) for hc in range(4)]
            attention(s, sts[s], filler)
            if debug:
                nc.sync.dma_start(out=dbg["qk_dbg"][s], in_=sts[s]["qk"])
                nc.sync.dma_start(out=dbg["v_dbg"][s], in_=sts[s]["v"])
                nc.sync.dma_start(out=dbg["ctxu_dbg"][s], in_=sts[s]["ctxu"])
                if s > 0:
                    nc.sync.dma_start(out=dbg["ctxs_dbg"][s - 1],
                                      in_=sts[s - 1]["ctx_s"])

        # ---------- epilogue: last segment normalize + outproj ----------
        SL = NSEG - 1
        stl = sts[SL]
        norm_mul(SL, 4, stl)
        norm_mul(SL, 5, stl)
        for half in (0, 1):
            fcs = [3 * half + k for k in range(3)]
            psos = [pp_proj.tile([P, SEG], F32, tag="proj", name=f"ep{half}_0"),
                    pp_proj.tile([P, SEG], F32, tag="proj", name=f"ep{half}_1"),
                    pp_cb.tile([P, SEG], F32, tag="cb", name=f"ep{half}_2")]
            for dc in range(DC):
                for k, fc in enumerate(fcs):
                    nc.tensor.matmul(
                        psos[k],
                        w_out_sb[:, dc, fc * P:(fc + 1) * P],
                        stl["ctx_s"][:, dc, :],
                        start=(dc == 0), stop=(dc == DC - 1))
            for k, fc in enumerate(fcs):
                ot = out_pool.tile([P, SEG], mm_dt, tag="ot", name=f"eo{half}_{fc}")
                # DVE drain; ACT would pay a 1.3us table load (Exp -> Identity)
                nc.vector.tensor_scalar_add(ot, psos[k], bout_sb[:, fc:fc + 1])
                nc.sync.dma_start(
                    out=outT[fc * P:(fc + 1) * P, SL * SEG:(SL + 1) * SEG], in_=ot)
        if debug:
            nc.sync.dma_start(out=dbg["ctxs_dbg"][SL], in_=stl["ctx_s"])

    nc.compile()
    return nc


def make_in_maps(x, Wqkv, bqkv, Wout, bout):
    """Shard full inputs across 8 cores: core = o*B + b."""
    r, E3, D = Wqkv.shape
    Bb, S, _ = x.shape
    DC = D // P
    in_maps = []
    for c in range(r * Bb):
        o, b = c // Bb, c % Bb
        in_maps.append({
            "xoT": np.ascontiguousarray(x[b, o::r, :].T).astype(ml_dtypes.bfloat16),
            "wqkvT": np.ascontiguousarray(Wqkv[o].T).astype(ml_dtypes.bfloat16),
            "woutT": np.ascontiguousarray(Wout[o].T).astype(ml_dtypes.bfloat16),
            "bqkv_pt": np.ascontiguousarray(bqkv[o].reshape(3 * DC, P).T),
            "bout_pt": np.ascontiguousarray(bout[o].reshape(DC, P).T),
            "bv": np.ascontiguousarray(bqkv[o, 2 * D:3 * D]),
        })
    return in_maps


_NC_CACHE = {}


def get_nc():
    if "nc" not in _NC_CACHE:
        _NC_CACHE["nc"] = build_nc()
    return _NC_CACHE["nc"]


def run(inputs, trace=False, **kwargs):
    """Run the SPMD kernel; returns (full_output, BassKernelResults)."""
    x = np.ascontiguousarray(np.asarray(inputs["x"], dtype=np.float32))
    Wqkv = np.asarray(inputs["Wqkv"], dtype=np.float32)
    bqkv = np.asarray(inputs["bqkv"], dtype=np.float32)
    Wout = np.asarray(inputs["Wout"], dtype=np.float32)
    bout = np.asarray(inputs["bout"], dtype=np.float32)
    r, E3, D = Wqkv.shape
    Bb, S, _ = x.shape

    nc = get_nc()
    in_maps = make_in_maps(x, Wqkv, bqkv, Wout, bout)
    res = run_bass_kernel_spmd(nc, in_maps, core_ids=list(range(len(in_maps))),
                               trace=trace, **kwargs)

    out = np.zeros((Bb, S, r * D), np.float32)
    for c in range(len(in_maps)):
        o, b = c // Bb, c % Bb
        out[b, o::r, o * D:(o + 1) * D] = \
            np.asarray(res.results[c]["outT"]).astype(np.float32).T
    return out, res


def kernel(x, Wqkv, bqkv, Wout, bout, num_heads):
    assert int(num_heads) == H0
    out, _ = run(dict(x=x, Wqkv=Wqkv, bqkv=bqkv, Wout=Wout, bout=bout))
    return out


# revision 37
# speedup vs baseline: 1.0911x; 1.0221x over previous
"""Blocksparse dilated attention TRN2 kernel (v2).

Sharding: 8 cores = r(=4 dilation offsets) x B(=2 batch). Each core runs one
independent per-offset attention branch on its strided token subset
(x[b, o::r, :]), with that offset's own weights. Host does the strided
gather (+transpose to channel-major) and the final scatter into the
zero-padded (B, S, r*D) output.

Per-core math (L=2048 tokens, D=768, H=12 heads, hd=64, segment=512):
  qkvT = Wqkv @ xoT            (channel-on-partition for q,k; token-major v)
  per (segment, head-pair):
    scoresT = kT-chunks.T x qT   (k on partitions; the two heads of a
          128-channel chunk occupy PE row-groups 0-63/64-127 and stream
          concurrently)
    attnT = exp(scale * scoresT) (ACT; no max-subtract: scores std ~0.3)
    ctxuT = [v_h0 | v_h1].T-packed: two col-tiled M=64 matmuls at array
          col-groups 0-1 / 2-3, streaming each head's attnT concurrently
          -> full 128-col PE utilization (v1 used M=65 = 51%).
  per head-quad w (denominator):
    den = onesT.T x attnT as four col-tiled M=32 matmuls (replicated ones
          columns) at col positions {0,32,64,96}, accumulated over key
          chunks -> one PSUM bank holds 4 heads' denominators.
    den -> (DVE drain) -> (SWDGE spread over 128 partitions) -> DVE
          reciprocal -> rc_dram  (DVE recip is ~6.5ns/elem/lane, hence
          the spread)
  normalize: rc broadcast per head-chunk via SWDGE partition-step-0 DMA
          (DRAM source required), then DVE mul into bf16 ctx_s.
  outT = Wout @ ctx_s + bout   (bf16 output; host upcasts)

Matmuls in bf16 (fp8 measured 5-25x over the 2e-2 error gate on host).

Emission fully software-pipelines across segments: attention(s)'s c-loop
interleaves filler tasks [proj(s+1), outproj(s-1), normalize muls] so the
PE never idles and the ACT exp stream (1.06us per [128,2,512] tile — the
per-iteration co-bottleneck) is hidden.  wout's DMA is gated behind a tiny
DVE memset so the startup HBM burst only carries first-needed bytes.
"""

import math
import sys
from contextlib import ExitStack

import ml_dtypes
import numpy as np

for _p in ("/opt/trn_rl_repo",):
    if _p not in sys.path:
        sys.path.insert(0, _p)

import concourse.bass as bass
import concourse.mybir as mybir
import concourse.tile as tile
from concourse import bacc
from concourse.bass_utils import run_bass_kernel_spmd

P = 128

# Problem constants (hardcoded per harness contract)
B0, S0, D0 = 2, 8192, 768
R0 = 4
H0, HD0 = 12, 64
SEG0 = 512
NSEG0 = (S0 // R0) // SEG0  # 4
N_CORES = 8

F32 = mybir.dt.float32
BF16 = mybir.dt.bfloat16


def build_nc(D=D0, H=H0, HD=HD0, SEG=SEG0, NSEG=NSEG0, mm_dt=BF16, debug=False):
    """Build the per-core Bass program (same NEFF on all cores)."""
    DC = D // P                # channel chunks of 128 (6)
    L = SEG * NSEG             # tokens per core (2048)
    KC = SEG // P              # key chunks per segment (4)
    HPC = P // HD              # heads per 128-channel chunk (2)
    NW = H // 4                # denominator head-quad windows per segment (3)
    E3 = 3 * D
    scale = 1.0 / math.sqrt(HD)
    assert D == H * HD and SEG % P == 0 and D % P == 0 and KC % 2 == 0

    nc = bacc.Bacc(trn_type="TRN2")
    xoT = nc.dram_tensor("xoT", [D, L], mm_dt, kind="ExternalInput")
    wqkvT = nc.dram_tensor("wqkvT", [D, E3], mm_dt, kind="ExternalInput")
    woutT = nc.dram_tensor("woutT", [D, D], mm_dt, kind="ExternalInput")
    bqkv_pt = nc.dram_tensor("bqkv_pt", [P, 3 * DC], F32, kind="ExternalInput")
    bout_pt = nc.dram_tensor("bout_pt", [P, DC], F32, kind="ExternalInput")
    bv = nc.dram_tensor("bv", [D], F32, kind="ExternalInput")
    outT = nc.dram_tensor("outT", [D, L], mm_dt, kind="ExternalOutput")
    # rc staging: the partition-step-0 broadcast DMA needs a DRAM source
    rc_dram = nc.dram_tensor("rc_dram", [NSEG, H * SEG], mm_dt,
                             kind="ExternalOutput" if debug else "Internal")
    if debug:
        dbg = {
            "qk_dbg": nc.dram_tensor("qk_dbg", [NSEG, P, 2 * (D // P), SEG],
                                     mm_dt, kind="ExternalOutput"),
            "v_dbg": nc.dram_tensor("v_dbg", [NSEG, P, SEG // P, D], mm_dt,
                                    kind="ExternalOutput"),
            "at_dbg": nc.dram_tensor("at_dbg", [NSEG, D // P, P, P // HD,
                                                SEG // P, SEG], mm_dt,
                                     kind="ExternalOutput"),
            "den_dbg": nc.dram_tensor("den_dbg", [NSEG, H // 4, P, SEG], F32,
                                      kind="ExternalOutput"),
            "ctxu_dbg": nc.dram_tensor("ctxu_dbg", [NSEG, P, D // P, SEG],
                                       mm_dt, kind="ExternalOutput"),
            "ctxs_dbg": nc.dram_tensor("ctxs_dbg", [NSEG, P, D // P, SEG],
                                       mm_dt, kind="ExternalOutput"),
        }

    with ExitStack() as ctx:
        tc = ctx.enter_context(tile.TileContext(nc))
        singles = ctx.enter_context(tc.tile_pool(name="singles", bufs=1))
        xo_pool = ctx.enter_context(tc.tile_pool(name="xo", bufs=2))
        qk_pool = ctx.enter_context(tc.tile_pool(name="qk", bufs=2))
        v_pool = ctx.enter_context(tc.tile_pool(name="v", bufs=2))
        at_pool = ctx.enter_context(tc.tile_pool(name="attn", bufs=3))
        ctxu_pool = ctx.enter_context(tc.tile_pool(name="ctxu", bufs=2))
        ctxs_pool = ctx.enter_context(tc.tile_pool(name="ctxs", bufs=3))
        out_pool = ctx.enter_context(tc.tile_pool(name="outp", bufs=4))
        bcast_pool = ctx.enter_context(tc.tile_pool(name="bcast", bufs=4))
        den_pool = ctx.enter_context(tc.tile_pool(name="den", bufs=2))
        pp_proj = ctx.enter_context(tc.tile_pool(name="pp_proj", bufs=2, space="PSUM"))
        pp_scA = ctx.enter_context(tc.tile_pool(name="pp_scA", bufs=1, space="PSUM"))
        pp_scB = ctx.enter_context(tc.tile_pool(name="pp_scB", bufs=1, space="PSUM"))
        pp_cb = ctx.enter_context(tc.tile_pool(name="pp_cb", bufs=2, space="PSUM"))

        # --- persistent biases / ones (tiny, first) ---
        bqkv_sb = singles.tile([P, 3 * DC], F32, tag="bqkv")
        nc.sync.dma_start(out=bqkv_sb, in_=bqkv_pt[:, :])
        bout_sb = singles.tile([P, DC], F32, tag="bout")
        nc.sync.dma_start(out=bout_sb, in_=bout_pt[:, :])
        bv_sb = singles.tile([P, D], F32, tag="bv")
        bv_ap = bv[:]
        bv_bcast = bass.AP(tensor=bv_ap.tensor, offset=bv_ap.offset,
                           ap=[[0, P], *bv_ap.ap])
        nc.gpsimd.dma_start(out=bv_sb, in_=bv_bcast)
        ones_sb = singles.tile([P, 1], mm_dt, tag="ones")
        nc.vector.memset(ones_sb, 1.0)

        # --- segment-0 inputs + q,k weight sections interleaved (these gate
        # the first matmuls; v next; wout deferred via a DVE gate) ---
        w_qkv_sb = singles.tile([P, DC, E3], mm_dt, tag="wqkv")
        xo_tiles = {}
        xo0 = xo_pool.tile([P, DC, SEG], mm_dt, tag="xo", name="xo_s0")
        xo_tiles[0] = xo0
        for dc in range(DC):
            nc.sync.dma_start(out=xo0[:, dc, :], in_=xoT[dc * P:(dc + 1) * P, 0:SEG])
            for sec in (0, 1):
                nc.sync.dma_start(
                    out=w_qkv_sb[:, dc, sec * D:(sec + 1) * D],
                    in_=wqkvT[dc * P:(dc + 1) * P, sec * D:(sec + 1) * D])
        for dc in range(DC):
            nc.sync.dma_start(
                out=w_qkv_sb[:, dc, 2 * D:3 * D],
                in_=wqkvT[dc * P:(dc + 1) * P, 2 * D:3 * D])
        w_out_sb = singles.tile([P, DC, D], mm_dt, tag="wout")

        def wout_task():
            # memset gate: the wout DMAs carry a WAW dep on this DVE op, so
            # they only start mid-attention(0) instead of burning startup HBM
            nc.vector.memset(w_out_sb[:, :, 0:1], 0.0)
            for dc in range(DC):
                nc.sync.dma_start(out=w_out_sb[:, dc, :],
                                  in_=woutT[dc * P:(dc + 1) * P, :])

        # ---------- projection tasks (qkv for one segment) ----------
        def make_xo_task(s):
            """Allocate segment-s xo tile + its load task (2-segment lead:
            the pool-WAR dep on the old slot's readers naturally delays the
            DMA past startup without stalling anything)."""
            xo_s = xo_pool.tile([P, DC, SEG], mm_dt, tag="xo", name=f"xo_s{s}")
            xo_tiles[s] = xo_s

            def xo_dma():
                for dc in range(DC):
                    nc.sync.dma_start(
                        out=xo_s[:, dc, :],
                        in_=xoT[dc * P:(dc + 1) * P, s * SEG:(s + 1) * SEG])
            return xo_dma

        def make_proj_tasks(s):
            """Allocate segment-s tiles; return (state, task list)."""
            st = {}
            if s in xo_tiles:
                xo_s = xo_tiles[s]
            else:
                xo_s = xo_pool.tile([P, DC, SEG], mm_dt, tag="xo", name=f"xo_s{s}")
                xo_tiles[s] = xo_s
            qk_s = qk_pool.tile([P, 2 * DC, SEG], mm_dt, tag="qk", name=f"qk_s{s}")
            v_s = v_pool.tile([P, KC, D], mm_dt, tag="v", name=f"v_s{s}")
            st["xo"], st["qk"], st["v"] = xo_s, qk_s, v_s

            def xo_task():
                # s=1 only: DVE memset gate defers the DMAs past the startup
                # HBM burst (DVE reaches it after the first proj drains)
                nc.vector.memset(xo_s[:, :, 0:1], 0.0)
                for dc in range(DC):
                    nc.sync.dma_start(
                        out=xo_s[:, dc, :],
                        in_=xoT[dc * P:(dc + 1) * P, s * SEG:(s + 1) * SEG])

            def qk_task(ec):
                ps = pp_proj.tile([P, SEG], F32, tag="proj", name=f"psqk{s}_{ec}")
                for dc in range(DC):
                    nc.tensor.matmul(
                        ps,
                        w_qkv_sb[:, dc, ec * P:(ec + 1) * P],
                        xo_s[:, dc, :],
                        start=(dc == 0), stop=(dc == DC - 1))
                nc.vector.tensor_scalar_add(qk_s[:, ec, :], ps, bqkv_sb[:, ec:ec + 1])

            def v_task(lc, n0, n):
                psv = pp_proj.tile([P, SEG], F32, tag="proj", name=f"psv{s}_{lc}_{n0}")
                for dc in range(DC):
                    nc.tensor.matmul(
                        psv[:, :n],
                        xo_s[:, dc, lc * P:(lc + 1) * P],
                        w_qkv_sb[:, dc, 2 * D + n0: 2 * D + n0 + n],
                        start=(dc == 0), stop=(dc == DC - 1))
                nc.vector.tensor_add(v_s[:, lc, n0:n0 + n], psv[:, :n],
                                     bv_sb[:, n0:n0 + n])

            v_list = [(lc, n0, min(512, D - n0))
                      for lc in range(KC) for n0 in range(0, D, 512)]
            if s == 0:
                # prologue: q groups then k then v, matching weight DMA arrival
                tasks = ([lambda ec=c: qk_task(ec) for c in range(2 * DC)]
                         + [lambda a=a: v_task(*a) for a in v_list])
            else:
                tasks = [xo_task] if s == 1 else []
                vi = 0
                for c in range(DC):
                    tasks.append(lambda ec=c: qk_task(ec))
                    tasks.append(lambda ec=DC + c: qk_task(ec))
                    if vi < len(v_list):
                        tasks.append(lambda a=v_list[vi]: v_task(*a))
                        vi += 1
                while vi < len(v_list):
                    tasks.append(lambda a=v_list[vi]: v_task(*a))
                    vi += 1
            return st, tasks

        # ---------- denominator chain + normalize ----------
        def den_chain(s, w, den_ps):
            """den_ps holds 4 heads' denominators (replicated x32 rows).
            Drain -> spread over 128 partitions -> reciprocal -> rc_dram."""
            den_sb = den_pool.tile([P, SEG], F32, tag="densb", name=f"dsb{s}_{w}")
            # ACT drain (Copy needs no activation table): keeps the chain off
            # the backlogged DVE queue
            nc.scalar.copy(den_sb, den_ps)
            if debug:
                nc.sync.dma_start(out=dbg["den_dbg"][s, w], in_=den_sb)
            den_t = den_pool.tile([P, 16], F32, tag="dent", name=f"dt{s}_{w}")
            for j in range(4):
                nc.sync.dma_start(out=den_t[32 * j:32 * (j + 1), :],
                                  in_=den_sb[32 * j:32 * j + 1, :])
            rc_t = den_pool.tile([P, 16], mm_dt, tag="rct", name=f"rt{s}_{w}")
            with nc.allow_low_precision(
                    reason="softmax denominator reciprocal; bf16 scale factor"):
                nc.vector.reciprocal(rc_t, den_t)
            nc.sync.dma_start(
                out=rc_dram[s:s + 1, 4 * w * SEG:(4 * w + 4) * SEG], in_=rc_t)

        def norm_chunk(s, hc, stt):
            """rc broadcast now; the DVE mul is deferred to the end of the
            segment's c-loop, when the bcast has long completed — a DVE mul
            emitted right here would wait ~2us on the SWDGE DMA and
            head-of-line-block the proj psum drains behind it in the DVE
            FIFO, stalling the PE past the 3.4us HAM window."""
            bcs = bcast_pool.tile([P, SEG], mm_dt, tag="bcs", name=f"bcs{s}_{hc}")
            rr = rc_dram[s:s + 1, hc * HPC * SEG:(hc + 1) * HPC * SEG]
            rr_b = bass.AP(tensor=rr.tensor, offset=rr.offset,
                           ap=[[SEG, HPC], [0, HD], [1, SEG]])
            nc.gpsimd.dma_start(out=bcs, in_=rr_b)
            stt["pending_muls"].append((hc, bcs))

        # ---------- output projection ----------
        def outproj_task(s, fc, stt):
            pso = pp_proj.tile([P, SEG], F32, tag="proj", name=f"pso{s}_{fc}")
            for dc in range(DC):
                nc.tensor.matmul(
                    pso,
                    w_out_sb[:, dc, fc * P:(fc + 1) * P],
                    stt["ctx_s"][:, dc, :],
                    start=(dc == 0), stop=(dc == DC - 1))
            ot = out_pool.tile([P, SEG], mm_dt, tag="ot", name=f"ot{s}_{fc}")
            nc.vector.tensor_scalar_add(ot, pso, bout_sb[:, fc:fc + 1])
            nc.sync.dma_start(
                out=outT[fc * P:(fc + 1) * P, s * SEG:(s + 1) * SEG], in_=ot)

        # ---------- attention c-loop ----------
        def attention(s, stt, filler):
            qk_s, v_s = stt["qk"], stt["v"]
            ctxu = ctxu_pool.tile([P, DC, SEG], mm_dt, tag="ctxu", name=f"cu{s}")
            ctx_s = ctxs_pool.tile([P, DC, SEG], mm_dt, tag="ctxs", name=f"cs{s}")
            stt["ctxu"], stt["ctx_s"] = ctxu, ctx_s
            stt["pending_muls"] = []
            ats = {}
            fi = 0
            npts = 3 * (DC + 1)

            def drain(pt):
                nonlocal fi
                want = min(len(filler), int(len(filler) * pt / npts + 0.5))
                while fi < want:
                    filler[fi]()
                    fi += 1

            def den_window(w):
                den_ps = pp_proj.tile([P, SEG], F32, tag="proj",
                                      name=f"dps{s}_{w}")
                for kc in range(KC):
                    for j in range(4):
                        h = 4 * w + j
                        nc.tensor.matmul(
                            den_ps[32 * j:32 * j + 1, :],
                            ones_sb,
                            ats[h // HPC][:, h % HPC, kc, :],
                            start=(kc == 0), stop=(kc == KC - 1),
                            tile_position=(0, 32 * j))
                den_chain(s, w, den_ps)
                norm_chunk(s, 2 * w, stt)
                norm_chunk(s, 2 * w + 1, stt)

            for c in range(DC + 1):
                # den window for a finished chunk pair, before this c's at2
                # alloc can recycle the buffers it reads
                if c in (3, 5):
                    den_window((c - 3) // 2)
                if c < DC:
                    at2 = at_pool.tile([P, HPC, KC, SEG], mm_dt, tag="attn",
                                       name=f"at{s}_{c}")
                    ats[c] = at2
                    for w in range(KC // 2):
                        for half, pool in ((0, pp_scA), (1, pp_scB)):
                            kc = 2 * w + half
                            sc = pool.tile([P, HPC, SEG], F32, tag=f"sc{half}",
                                           name=f"sc{half}_{s}_{c}_{w}")
                            for i in range(HPC):
                                ho = i * HD
                                nc.tensor.matmul(
                                    sc[:, i, :],
                                    qk_s[ho:ho + HD, DC + c, kc * P:(kc + 1) * P],
                                    qk_s[ho:ho + HD, c, :])
                            nc.scalar.activation(
                                at2[:, :, kc, :], sc,
                                mybir.ActivationFunctionType.Exp,
                                scale=scale)
                drain(3 * c + 1)
                if c == DC:
                    # last den window first: its reciprocal chain is the
                    # epilogue critical path, start it before ctx(DC-1)
                    den_window(2)
                if c > 0:
                    cp = c - 1
                    at2 = ats[cp]
                    cps = pp_cb.tile([P, SEG], F32, tag="cb", name=f"cps{s}_{cp}")
                    for kc in range(KC):
                        for i in range(HPC):
                            h = cp * HPC + i
                            nc.tensor.matmul(
                                cps[i * HD:(i + 1) * HD, :],
                                v_s[:, kc, h * HD:(h + 1) * HD],
                                at2[:, i, kc, :],
                                start=(kc == 0), stop=(kc == KC - 1),
                                tile_position=(0, i * HD))
                    nc.vector.tensor_copy(ctxu[:, cp, :], cps)
                    if debug:
                        nc.sync.dma_start(out=dbg["at_dbg"][s, cp],
                                          in_=at2[:, :, :, :])
                drain(3 * c + 2)
                if c >= 3:
                    ats.pop(c - 3, None)
                drain(3 * c + 3)
            for hc, bcs in stt["pending_muls"]:
                nc.vector.tensor_mul(ctx_s[:, hc, :], ctxu[:, hc, :], bcs)

        # ---------- main pipeline ----------
        sts = {}
        sts[0], tasks0 = make_proj_tasks(0)
        for t in tasks0:
            t()
        for s in range(NSEG):
            nxt = s + 1
            if nxt < NSEG:
                sts[nxt], proj_tasks = make_proj_tasks(nxt)
            else:
                proj_tasks = []
            filler = []
            if s + 2 < NSEG:
                filler.append(make_xo_task(s + 2))
            # prev-segment outproj, rebalanced so the proj-less attention(3)
            # keeps enough PE filler to never idle past the 3.4us HAM window
            if s == 1:
                op_specs = [(0, fc) for fc in range(DC)]
            elif s == 2:
                op_specs = [(1, fc) for fc in range(3)]
            elif s == 3:
                op_specs = ([(1, fc) for fc in range(3, DC)]
                            + [(2, fc) for fc in range(DC)])
            else:
                op_specs = []
            op_tasks = [(lambda a=a, fc=fc: outproj_task(a, fc, sts[a]))
                        for a, fc in op_specs]
            merged = []
            pi = oi = 0
            for k in range(len(proj_tasks) + len(op_tasks)):
                take_op = (oi < len(op_tasks)
                           and (k >= 4 and (k - 4) % 4 == 3 or pi >= len(proj_tasks)))
                if take_op:
                    merged.append(op_tasks[oi])
                    oi += 1
                else:
                    merged.append(proj_tasks[pi])
                    pi += 1
            filler += merged
            if s == 0:
                filler.insert(2, wout_task)
            attention(s, sts[s], filler)
            if debug:
                nc.sync.dma_start(out=dbg["qk_dbg"][s], in_=sts[s]["qk"])
                nc.sync.dma_start(out=dbg["v_dbg"][s], in_=sts[s]["v"])
                nc.sync.dma_start(out=dbg["ctxu_dbg"][s], in_=sts[s]["ctxu"])
                if s > 0:
                    nc.sync.dma_start(out=dbg["ctxs_dbg"][s - 1],
                                      in_=sts[s - 1]["ctx_s"])

        # ---------- epilogue: last segment outproj (normalize ran in-loop) ---
        SL = NSEG - 1
        stl = sts[SL]
        for half in (0, 1):
            fcs = [3 * half + k for k in range(3)]
            psos = [pp_proj.tile([P, SEG], F32, tag="proj", name=f"ep{half}_0"),
                    pp_proj.tile([P, SEG], F32, tag="proj", name=f"ep{half}_1"),
                    pp_cb.tile([P, SEG], F32, tag="cb", name=f"ep{half}_2")]
            for dc in range(DC):
                for k, fc in enumerate(fcs):
                    nc.tensor.matmul(
                        psos[k],
                        w_out_sb[:, dc, fc * P:(fc + 1) * P],
                        stl["ctx_s"][:, dc, :],
                        start=(dc == 0), stop=(dc == DC - 1))
            for k, fc in enumerate(fcs):
                ot = out_pool.tile([P, SEG], mm_dt, tag="ot", name=f"eo{half}_{fc}")
                # DVE drain; ACT would pay a 1.3us table load (Exp -> Identity)
                nc.vector.tensor_scalar_add(ot, psos[k], bout_sb[:, fc:fc + 1])
                nc.sync.dma_start(
                    out=outT[fc * P:(fc + 1) * P, SL * SEG:(SL + 1) * SEG], in_=ot)
        if debug:
            nc.sync.dma_start(out=dbg["ctxs_dbg"][SL], in_=stl["ctx_s"])

    nc.compile()
    return nc


def make_in_maps(x, Wqkv, bqkv, Wout, bout):
    """Shard full inputs across 8 cores: core = o*B + b."""
    r, E3, D = Wqkv.shape
    Bb, S, _ = x.shape
    DC = D // P
    in_maps = []
    for c in range(r * Bb):
        o, b = c // Bb, c % Bb
        in_maps.append({
            "xoT": np.ascontiguousarray(x[b, o::r, :].T).astype(ml_dtypes.bfloat16),
            "wqkvT": np.ascontiguousarray(Wqkv[o].T).astype(ml_dtypes.bfloat16),
            "woutT": np.ascontiguousarray(Wout[o].T).astype(ml_dtypes.bfloat16),
            "bqkv_pt": np.ascontiguousarray(bqkv[o].reshape(3 * DC, P).T),
            "bout_pt": np.ascontiguousarray(bout[o].reshape(DC, P).T),
            "bv": np.ascontiguousarray(bqkv[o, 2 * D:3 * D]),
        })
    return in_maps


_NC_CACHE = {}


def get_nc():
    if "nc" not in _NC_CACHE:
        _NC_CACHE["nc"] = build_nc()
    return _NC_CACHE["nc"]


def run(inputs, trace=False, **kwargs):
    """Run the SPMD kernel; returns (full_output, BassKernelResults)."""
    x = np.ascontiguousarray(np.asarray(inputs["x"], dtype=np.float32))
    Wqkv = np.asarray(inputs["Wqkv"], dtype=np.float32)
    bqkv = np.asarray(inputs["bqkv"], dtype=np.float32)
    Wout = np.asarray(inputs["Wout"], dtype=np.float32)
    bout = np.asarray(inputs["bout"], dtype=np.float32)
    r, E3, D = Wqkv.shape
    Bb, S, _ = x.shape

    nc = get_nc()
    in_maps = make_in_maps(x, Wqkv, bqkv, Wout, bout)
    res = run_bass_kernel_spmd(nc, in_maps, core_ids=list(range(len(in_maps))),
                               trace=trace, **kwargs)

    out = np.zeros((Bb, S, r * D), np.float32)
    for c in range(len(in_maps)):
        o, b = c // Bb, c % Bb
        out[b, o::r, o * D:(o + 1) * D] = \
            np.asarray(res.results[c]["outT"]).astype(np.float32).T
    return out, res


def kernel(x, Wqkv, bqkv, Wout, bout, num_heads):
    assert int(num_heads) == H0
    out, _ = run(dict(x=x, Wqkv=Wqkv, bqkv=bqkv, Wout=Wout, bout=bout))
    return out


# revision 42
# speedup vs baseline: 1.1123x; 1.0194x over previous
"""Blocksparse dilated attention TRN2 kernel (v2).

Sharding: 8 cores = r(=4 dilation offsets) x B(=2 batch). Each core runs one
independent per-offset attention branch on its strided token subset
(x[b, o::r, :]), with that offset's own weights. Host does the strided
gather (+transpose to channel-major) and the final scatter into the
zero-padded (B, S, r*D) output.

Per-core math (L=2048 tokens, D=768, H=12 heads, hd=64, segment=512):
  qkvT = Wqkv @ xoT            (channel-on-partition for q,k; token-major v)
  per (segment, head-pair):
    scoresT = kT-chunks.T x qT   (k on partitions; the two heads of a
          128-channel chunk occupy PE row-groups 0-63/64-127 and stream
          concurrently)
    attnT = exp(scale * scoresT) (ACT; no max-subtract: scores std ~0.3)
    ctxuT = [v_h0 | v_h1].T-packed: two col-tiled M=64 matmuls at array
          col-groups 0-1 / 2-3, streaming each head's attnT concurrently
          -> full 128-col PE utilization (v1 used M=65 = 51%).
  per head-quad w (denominator):
    den = onesT.T x attnT as four col-tiled M=32 matmuls (replicated ones
          columns) at col positions {0,32,64,96}, accumulated over key
          chunks -> one PSUM bank holds 4 heads' denominators.
    den -> (DVE drain) -> (SWDGE spread over 128 partitions) -> DVE
          reciprocal -> rc_dram  (DVE recip is ~6.5ns/elem/lane, hence
          the spread)
  normalize: rc broadcast per head-chunk via SWDGE partition-step-0 DMA
          (DRAM source required), then DVE mul into bf16 ctx_s.
  outT = Wout @ ctx_s + bout   (bf16 output; host upcasts)

Matmuls in bf16 (fp8 measured 5-25x over the 2e-2 error gate on host).

Emission fully software-pipelines across segments: attention(s)'s c-loop
interleaves filler tasks [proj(s+1), outproj(s-1), normalize muls] so the
PE never idles and the ACT exp stream (1.06us per [128,2,512] tile — the
per-iteration co-bottleneck) is hidden.  wout's DMA is gated behind a tiny
DVE memset so the startup HBM burst only carries first-needed bytes.
"""

import math
import sys
from contextlib import ExitStack

import ml_dtypes
import numpy as np

for _p in ("/opt/trn_rl_repo",):
    if _p not in sys.path:
        sys.path.insert(0, _p)

import concourse.bass as bass
import concourse.mybir as mybir
import concourse.tile as tile
from concourse import bacc
from concourse.bass_utils import run_bass_kernel_spmd

P = 128

# Problem constants (hardcoded per harness contract)
B0, S0, D0 = 2, 8192, 768
R0 = 4
H0, HD0 = 12, 64
SEG0 = 512
NSEG0 = (S0 // R0) // SEG0  # 4
N_CORES = 8

F32 = mybir.dt.float32
BF16 = mybir.dt.bfloat16


def build_nc(D=D0, H=H0, HD=HD0, SEG=SEG0, NSEG=NSEG0, mm_dt=BF16, debug=False):
    """Build the per-core Bass program (same NEFF on all cores)."""
    DC = D // P                # channel chunks of 128 (6)
    L = SEG * NSEG             # tokens per core (2048)
    KC = SEG // P              # key chunks per segment (4)
    HPC = P // HD              # heads per 128-channel chunk (2)
    NW = H // 4                # denominator head-quad windows per segment (3)
    E3 = 3 * D
    scale = 1.0 / math.sqrt(HD)
    assert D == H * HD and SEG % P == 0 and D % P == 0 and KC % 2 == 0

    nc = bacc.Bacc(trn_type="TRN2")
    xoT = nc.dram_tensor("xoT", [D, L], mm_dt, kind="ExternalInput")
    wqkvT = nc.dram_tensor("wqkvT", [D, E3], mm_dt, kind="ExternalInput")
    woutT = nc.dram_tensor("woutT", [D, D], mm_dt, kind="ExternalInput")
    bqkv_pt = nc.dram_tensor("bqkv_pt", [P, 3 * DC], F32, kind="ExternalInput")
    bout_pt = nc.dram_tensor("bout_pt", [P, DC], F32, kind="ExternalInput")
    bv = nc.dram_tensor("bv", [D], F32, kind="ExternalInput")
    outT = nc.dram_tensor("outT", [D, L], mm_dt, kind="ExternalOutput")
    # rc staging: the partition-step-0 broadcast DMA needs a DRAM source
    rc_dram = nc.dram_tensor("rc_dram", [NSEG, H * SEG], mm_dt,
                             kind="ExternalOutput" if debug else "Internal")
    if debug:
        dbg = {
            "qk_dbg": nc.dram_tensor("qk_dbg", [NSEG, P, 2 * (D // P), SEG],
                                     mm_dt, kind="ExternalOutput"),
            "v_dbg": nc.dram_tensor("v_dbg", [NSEG, P, SEG // P, D], mm_dt,
                                    kind="ExternalOutput"),
            "at_dbg": nc.dram_tensor("at_dbg", [NSEG, D // P, P, P // HD,
                                                SEG // P, SEG], mm_dt,
                                     kind="ExternalOutput"),
            "den_dbg": nc.dram_tensor("den_dbg", [NSEG, H // 4, P, SEG], F32,
                                      kind="ExternalOutput"),
            "ctxu_dbg": nc.dram_tensor("ctxu_dbg", [NSEG, P, D // P, SEG],
                                       mm_dt, kind="ExternalOutput"),
            "ctxs_dbg": nc.dram_tensor("ctxs_dbg", [NSEG, P, D // P, SEG],
                                       mm_dt, kind="ExternalOutput"),
        }

    with ExitStack() as ctx:
        tc = ctx.enter_context(tile.TileContext(nc))
        singles = ctx.enter_context(tc.tile_pool(name="singles", bufs=1))
        xo_pool = ctx.enter_context(tc.tile_pool(name="xo", bufs=2))
        qk_pool = ctx.enter_context(tc.tile_pool(name="qk", bufs=2))
        v_pool = ctx.enter_context(tc.tile_pool(name="v", bufs=2))
        at_pool = ctx.enter_context(tc.tile_pool(name="attn", bufs=3))
        ctxu_pool = ctx.enter_context(tc.tile_pool(name="ctxu", bufs=2))
        ctxs_pool = ctx.enter_context(tc.tile_pool(name="ctxs", bufs=3))
        out_pool = ctx.enter_context(tc.tile_pool(name="outp", bufs=4))
        bcast_pool = ctx.enter_context(tc.tile_pool(name="bcast", bufs=4))
        den_pool = ctx.enter_context(tc.tile_pool(name="den", bufs=2))
        pp_proj = ctx.enter_context(tc.tile_pool(name="pp_proj", bufs=2, space="PSUM"))
        pp_scA = ctx.enter_context(tc.tile_pool(name="pp_scA", bufs=1, space="PSUM"))
        pp_scB = ctx.enter_context(tc.tile_pool(name="pp_scB", bufs=1, space="PSUM"))
        pp_cb = ctx.enter_context(tc.tile_pool(name="pp_cb", bufs=2, space="PSUM"))

        # --- persistent biases / ones (tiny, first) ---
        bqkv_sb = singles.tile([P, 3 * DC], F32, tag="bqkv")
        nc.sync.dma_start(out=bqkv_sb, in_=bqkv_pt[:, :])
        bout_sb = singles.tile([P, DC], F32, tag="bout")
        nc.sync.dma_start(out=bout_sb, in_=bout_pt[:, :])
        bv_sb = singles.tile([P, D], F32, tag="bv")
        bv_ap = bv[:]
        bv_bcast = bass.AP(tensor=bv_ap.tensor, offset=bv_ap.offset,
                           ap=[[0, P], *bv_ap.ap])
        nc.gpsimd.dma_start(out=bv_sb, in_=bv_bcast)
        ones_sb = singles.tile([P, 1], mm_dt, tag="ones")
        nc.vector.memset(ones_sb, 1.0)

        # --- segment-0 inputs + q,k weight sections interleaved (these gate
        # the first matmuls; v next; wout deferred via a DVE gate) ---
        w_qkv_sb = singles.tile([P, DC, E3], mm_dt, tag="wqkv")
        xo_tiles = {}
        xo0 = xo_pool.tile([P, DC, SEG], mm_dt, tag="xo", name="xo_s0")
        xo_tiles[0] = xo0
        for dc in range(DC):
            nc.sync.dma_start(out=xo0[:, dc, :], in_=xoT[dc * P:(dc + 1) * P, 0:SEG])
            for sec in (0, 1):
                nc.sync.dma_start(
                    out=w_qkv_sb[:, dc, sec * D:(sec + 1) * D],
                    in_=wqkvT[dc * P:(dc + 1) * P, sec * D:(sec + 1) * D])
        for dc in range(DC):
            nc.sync.dma_start(
                out=w_qkv_sb[:, dc, 2 * D:3 * D],
                in_=wqkvT[dc * P:(dc + 1) * P, 2 * D:3 * D])
        w_out_sb = singles.tile([P, DC, D], mm_dt, tag="wout")

        def wout_task():
            # memset gate: the wout DMAs carry a WAW dep on this DVE op, so
            # they only start mid-attention(0) instead of burning startup HBM
            nc.vector.memset(w_out_sb[:, :, 0:1], 0.0)
            for dc in range(DC):
                nc.sync.dma_start(out=w_out_sb[:, dc, :],
                                  in_=woutT[dc * P:(dc + 1) * P, :])

        # ---------- projection tasks (qkv for one segment) ----------
        def make_xo_task(s):
            """Allocate segment-s xo tile + its load task (2-segment lead:
            the pool-WAR dep on the old slot's readers naturally delays the
            DMA past startup without stalling anything)."""
            xo_s = xo_pool.tile([P, DC, SEG], mm_dt, tag="xo", name=f"xo_s{s}")
            xo_tiles[s] = xo_s

            def xo_dma():
                for dc in range(DC):
                    nc.sync.dma_start(
                        out=xo_s[:, dc, :],
                        in_=xoT[dc * P:(dc + 1) * P, s * SEG:(s + 1) * SEG])
            return xo_dma

        def make_proj_tasks(s):
            """Allocate segment-s tiles; return (state, task list)."""
            st = {}
            if s in xo_tiles:
                xo_s = xo_tiles[s]
            else:
                xo_s = xo_pool.tile([P, DC, SEG], mm_dt, tag="xo", name=f"xo_s{s}")
                xo_tiles[s] = xo_s
            qk_s = qk_pool.tile([P, 2 * DC, SEG], mm_dt, tag="qk", name=f"qk_s{s}")
            v_s = v_pool.tile([P, KC, D], mm_dt, tag="v", name=f"v_s{s}")
            st["xo"], st["qk"], st["v"] = xo_s, qk_s, v_s

            def xo_task():
                # s=1 only: DVE memset gate defers the DMAs past the startup
                # HBM burst (DVE reaches it after the first proj drains)
                nc.vector.memset(xo_s[:, :, 0:1], 0.0)
                for dc in range(DC):
                    nc.sync.dma_start(
                        out=xo_s[:, dc, :],
                        in_=xoT[dc * P:(dc + 1) * P, s * SEG:(s + 1) * SEG])

            def qk_task(ec):
                ps = pp_proj.tile([P, SEG], F32, tag="proj", name=f"psqk{s}_{ec}")
                for dc in range(DC):
                    nc.tensor.matmul(
                        ps,
                        w_qkv_sb[:, dc, ec * P:(ec + 1) * P],
                        xo_s[:, dc, :],
                        start=(dc == 0), stop=(dc == DC - 1))
                nc.vector.tensor_scalar_add(qk_s[:, ec, :], ps, bqkv_sb[:, ec:ec + 1])

            def v_task(lc, n0, n):
                psv = pp_proj.tile([P, SEG], F32, tag="proj", name=f"psv{s}_{lc}_{n0}")
                for dc in range(DC):
                    nc.tensor.matmul(
                        psv[:, :n],
                        xo_s[:, dc, lc * P:(lc + 1) * P],
                        w_qkv_sb[:, dc, 2 * D + n0: 2 * D + n0 + n],
                        start=(dc == 0), stop=(dc == DC - 1))
                nc.vector.tensor_add(v_s[:, lc, n0:n0 + n], psv[:, :n],
                                     bv_sb[:, n0:n0 + n])

            v_list = [(lc, n0, min(512, D - n0))
                      for lc in range(KC) for n0 in range(0, D, 512)]
            if s == 0:
                # prologue: q groups then k then v, matching weight DMA arrival
                tasks = ([lambda ec=c: qk_task(ec) for c in range(2 * DC)]
                         + [lambda a=a: v_task(*a) for a in v_list])
            else:
                tasks = [xo_task] if s == 1 else []
                vi = 0
                for c in range(DC):
                    tasks.append(lambda ec=c: qk_task(ec))
                    tasks.append(lambda ec=DC + c: qk_task(ec))
                    if vi < len(v_list):
                        tasks.append(lambda a=v_list[vi]: v_task(*a))
                        vi += 1
                while vi < len(v_list):
                    tasks.append(lambda a=v_list[vi]: v_task(*a))
                    vi += 1
            return st, tasks

        # ---------- denominator chain + normalize ----------
        def den_chain(s, w, den_ps):
            """den_ps holds 4 heads' denominators (replicated x32 rows).
            Drain -> spread over 128 partitions -> reciprocal -> rc_dram."""
            den_sb = den_pool.tile([P, SEG], F32, tag="densb", name=f"dsb{s}_{w}")
            # ACT drain (Copy needs no activation table): keeps the chain off
            # the backlogged DVE queue
            nc.scalar.copy(den_sb, den_ps)
            if debug:
                nc.sync.dma_start(out=dbg["den_dbg"][s, w], in_=den_sb)
            den_t = den_pool.tile([P, 16], F32, tag="dent", name=f"dt{s}_{w}")
            for j in range(4):
                nc.sync.dma_start(out=den_t[32 * j:32 * (j + 1), :],
                                  in_=den_sb[32 * j:32 * j + 1, :])
            rc_t = den_pool.tile([P, 16], mm_dt, tag="rct", name=f"rt{s}_{w}")
            with nc.allow_low_precision(
                    reason="softmax denominator reciprocal; bf16 scale factor"):
                nc.vector.reciprocal(rc_t, den_t)
            nc.sync.dma_start(
                out=rc_dram[s:s + 1, 4 * w * SEG:(4 * w + 4) * SEG], in_=rc_t)

        def norm_chunk(s, hc, stt):
            """rc broadcast now; the DVE mul is deferred to the end of the
            segment's c-loop, when the bcast has long completed — a DVE mul
            emitted right here would wait ~2us on the SWDGE DMA and
            head-of-line-block the proj psum drains behind it in the DVE
            FIFO, stalling the PE past the 3.4us HAM window."""
            bcs = bcast_pool.tile([P, SEG], mm_dt, tag="bcs", name=f"bcs{s}_{hc}")
            rr = rc_dram[s:s + 1, hc * HPC * SEG:(hc + 1) * HPC * SEG]
            rr_b = bass.AP(tensor=rr.tensor, offset=rr.offset,
                           ap=[[SEG, HPC], [0, HD], [1, SEG]])
            nc.gpsimd.dma_start(out=bcs, in_=rr_b)
            stt["bcs"][hc] = bcs

        def norm_mul(s, hc, stt):
            nc.vector.tensor_mul(stt["ctx_s"][:, hc, :], stt["ctxu"][:, hc, :],
                                 stt["bcs"].pop(hc))

        # ---------- output projection ----------
        def outproj_task(s, fc, stt):
            pso = pp_proj.tile([P, SEG], F32, tag="proj", name=f"pso{s}_{fc}")
            for dc in range(DC):
                nc.tensor.matmul(
                    pso,
                    w_out_sb[:, dc, fc * P:(fc + 1) * P],
                    stt["ctx_s"][:, dc, :],
                    start=(dc == 0), stop=(dc == DC - 1))
            ot = out_pool.tile([P, SEG], mm_dt, tag="ot", name=f"ot{s}_{fc}")
            nc.vector.tensor_scalar_add(ot, pso, bout_sb[:, fc:fc + 1])
            nc.sync.dma_start(
                out=outT[fc * P:(fc + 1) * P, s * SEG:(s + 1) * SEG], in_=ot)

        # ---------- attention c-loop ----------
        def attention(s, stt, filler):
            qk_s, v_s = stt["qk"], stt["v"]
            ctxu = ctxu_pool.tile([P, DC, SEG], mm_dt, tag="ctxu", name=f"cu{s}")
            ctx_s = ctxs_pool.tile([P, DC, SEG], mm_dt, tag="ctxs", name=f"cs{s}")
            stt["ctxu"], stt["ctx_s"] = ctxu, ctx_s
            stt["bcs"] = {}
            ats = {}
            fi = 0
            npts = 3 * (DC + 1)

            def drain(pt):
                nonlocal fi
                want = min(len(filler), int(len(filler) * pt / npts + 0.5))
                while fi < want:
                    filler[fi]()
                    fi += 1

            def den_window(w):
                den_ps = pp_proj.tile([P, SEG], F32, tag="proj",
                                      name=f"dps{s}_{w}")
                for kc in range(KC):
                    for j in range(4):
                        h = 4 * w + j
                        nc.tensor.matmul(
                            den_ps[32 * j:32 * j + 1, :],
                            ones_sb,
                            ats[h // HPC][:, h % HPC, kc, :],
                            start=(kc == 0), stop=(kc == KC - 1),
                            tile_position=(0, 32 * j))
                den_chain(s, w, den_ps)
                norm_chunk(s, 2 * w, stt)
                norm_chunk(s, 2 * w + 1, stt)

            for c in range(DC + 1):
                # den window for a finished chunk pair, before this c's at2
                # alloc can recycle the buffers it reads
                if c in (3, 5):
                    den_window((c - 3) // 2)
                if c < DC:
                    at2 = at_pool.tile([P, HPC, KC, SEG], mm_dt, tag="attn",
                                       name=f"at{s}_{c}")
                    ats[c] = at2
                    for w in range(KC // 2):
                        for half, pool in ((0, pp_scA), (1, pp_scB)):
                            kc = 2 * w + half
                            sc = pool.tile([P, HPC, SEG], F32, tag=f"sc{half}",
                                           name=f"sc{half}_{s}_{c}_{w}")
                            for i in range(HPC):
                                ho = i * HD
                                nc.tensor.matmul(
                                    sc[:, i, :],
                                    qk_s[ho:ho + HD, DC + c, kc * P:(kc + 1) * P],
                                    qk_s[ho:ho + HD, c, :])
                            nc.scalar.activation(
                                at2[:, :, kc, :], sc,
                                mybir.ActivationFunctionType.Exp,
                                scale=scale)
                drain(3 * c + 1)
                if c == DC:
                    # last den window first: its reciprocal chain is the
                    # epilogue critical path, start it before ctx(DC-1)
                    den_window(2)
                if c > 0:
                    cp = c - 1
                    at2 = ats[cp]
                    cps = pp_cb.tile([P, SEG], F32, tag="cb", name=f"cps{s}_{cp}")
                    for kc in range(KC):
                        for i in range(HPC):
                            h = cp * HPC + i
                            nc.tensor.matmul(
                                cps[i * HD:(i + 1) * HD, :],
                                v_s[:, kc, h * HD:(h + 1) * HD],
                                at2[:, i, kc, :],
                                start=(kc == 0), stop=(kc == KC - 1),
                                tile_position=(0, i * HD))
                    nc.vector.tensor_copy(ctxu[:, cp, :], cps)
                    if debug:
                        nc.sync.dma_start(out=dbg["at_dbg"][s, cp],
                                          in_=at2[:, :, :, :])
                drain(3 * c + 2)
                if c >= 3:
                    ats.pop(c - 3, None)
                drain(3 * c + 3)
            # hc 0..3's broadcasts are long done — no DVE wait; hc 4/5 (whose
            # bcast is still in flight) are deferred into the next segment's
            # filler so they never head-of-line-block the boundary drains
            for hc in range(4):
                norm_mul(s, hc, stt)

        # ---------- main pipeline ----------
        sts = {}
        sts[0], tasks0 = make_proj_tasks(0)
        for t in tasks0:
            t()
        for s in range(NSEG):
            nxt = s + 1
            if nxt < NSEG:
                sts[nxt], proj_tasks = make_proj_tasks(nxt)
            else:
                proj_tasks = []
            filler = []
            if s + 2 < NSEG:
                filler.append(make_xo_task(s + 2))
            pre, proj_tasks = proj_tasks[:4], proj_tasks[4:]
            filler += pre
            if s > 0:
                filler.append(lambda a=(s - 1): norm_mul(a, 4, sts[a]))
                filler.append(lambda a=(s - 1): norm_mul(a, 5, sts[a]))
            # prev-segment outproj, rebalanced so the proj-less attention(3)
            # keeps enough PE filler to never idle past the 3.4us HAM window
            if s == 1:
                op_specs = [(0, fc) for fc in range(DC)]
            elif s == 2:
                op_specs = [(1, fc) for fc in range(3)]
            elif s == 3:
                op_specs = ([(1, fc) for fc in range(3, DC)]
                            + [(2, fc) for fc in range(DC)])
            else:
                op_specs = []
            op_tasks = [(lambda a=a, fc=fc: outproj_task(a, fc, sts[a]))
                        for a, fc in op_specs]
            merged = []
            pi = oi = 0
            for k in range(len(proj_tasks) + len(op_tasks)):
                take_op = (oi < len(op_tasks)
                           and (k >= 4 and (k - 4) % 4 == 3 or pi >= len(proj_tasks)))
                if take_op:
                    merged.append(op_tasks[oi])
                    oi += 1
                else:
                    merged.append(proj_tasks[pi])
                    pi += 1
            filler += merged
            if s == 0:
                filler.insert(2, wout_task)
            attention(s, sts[s], filler)
            if debug:
                nc.sync.dma_start(out=dbg["qk_dbg"][s], in_=sts[s]["qk"])
                nc.sync.dma_start(out=dbg["v_dbg"][s], in_=sts[s]["v"])
                nc.sync.dma_start(out=dbg["ctxu_dbg"][s], in_=sts[s]["ctxu"])
                if s > 0:
                    nc.sync.dma_start(out=dbg["ctxs_dbg"][s - 1],
                                      in_=sts[s - 1]["ctx_s"])

        # ---------- epilogue: last segment outproj ----------
        # Split the contraction: dc 0..3 partials run ungated while the last
        # den/normalize chain (hc 4,5) is still in flight, then a short
        # completion pass over dc 4,5. bout is folded into the partial drain.
        SL = NSEG - 1
        stl = sts[SL]
        partial = ctxu_pool.tile([P, DC, SEG], mm_dt, tag="ctxu", name="partial")

        def ep_psos(half, phase):
            return [pp_proj.tile([P, SEG], F32, tag="proj", name=f"ep{phase}{half}0"),
                    pp_proj.tile([P, SEG], F32, tag="proj", name=f"ep{phase}{half}1"),
                    pp_cb.tile([P, SEG], F32, tag="cb", name=f"ep{phase}{half}2")]

        for half in (0, 1):
            fcs = [3 * half + k for k in range(3)]
            psos = ep_psos(half, "a")
            for dc in range(4):
                for k, fc in enumerate(fcs):
                    nc.tensor.matmul(
                        psos[k],
                        w_out_sb[:, dc, fc * P:(fc + 1) * P],
                        stl["ctx_s"][:, dc, :],
                        start=(dc == 0), stop=(dc == 3))
            for k, fc in enumerate(fcs):
                nc.vector.tensor_scalar_add(partial[:, fc, :], psos[k],
                                            bout_sb[:, fc:fc + 1])
        norm_mul(SL, 4, stl)
        norm_mul(SL, 5, stl)
        for half in (0, 1):
            fcs = [3 * half + k for k in range(3)]
            psos = ep_psos(half, "c")
            for dc in (4, 5):
                for k, fc in enumerate(fcs):
                    nc.tensor.matmul(
                        psos[k],
                        w_out_sb[:, dc, fc * P:(fc + 1) * P],
                        stl["ctx_s"][:, dc, :],
                        start=(dc == 4), stop=(dc == 5))
            for k, fc in enumerate(fcs):
                ot = out_pool.tile([P, SEG], mm_dt, tag="ot", name=f"eo{half}_{fc}")
                nc.vector.tensor_add(ot, partial[:, fc, :], psos[k])
                nc.sync.dma_start(
                    out=outT[fc * P:(fc + 1) * P, SL * SEG:(SL + 1) * SEG], in_=ot)
        if debug:
            nc.sync.dma_start(out=dbg["ctxs_dbg"][SL], in_=stl["ctx_s"])

    nc.compile()
    return nc


def make_in_maps(x, Wqkv, bqkv, Wout, bout):
    """Shard full inputs across 8 cores: core = o*B + b."""
    r, E3, D = Wqkv.shape
    Bb, S, _ = x.shape
    DC = D // P
    in_maps = []
    for c in range(r * Bb):
        o, b = c // Bb, c % Bb
        in_maps.append({
            "xoT": np.ascontiguousarray(x[b, o::r, :].T).astype(ml_dtypes.bfloat16),
            "wqkvT": np.ascontiguousarray(Wqkv[o].T).astype(ml_dtypes.bfloat16),
            "woutT": np.ascontiguousarray(Wout[o].T).astype(ml_dtypes.bfloat16),
            "bqkv_pt": np.ascontiguousarray(bqkv[o].reshape(3 * DC, P).T),
            "bout_pt": np.ascontiguousarray(bout[o].reshape(DC, P).T),
            "bv": np.ascontiguousarray(bqkv[o, 2 * D:3 * D]),
        })
    return in_maps


_NC_CACHE = {}


def get_nc():
    if "nc" not in _NC_CACHE:
        _NC_CACHE["nc"] = build_nc()
    return _NC_CACHE["nc"]


def run(inputs, trace=False, **kwargs):
    """Run the SPMD kernel; returns (full_output, BassKernelResults)."""
    x = np.ascontiguousarray(np.asarray(inputs["x"], dtype=np.float32))
    Wqkv = np.asarray(inputs["Wqkv"], dtype=np.float32)
    bqkv = np.asarray(inputs["bqkv"], dtype=np.float32)
    Wout = np.asarray(inputs["Wout"], dtype=np.float32)
    bout = np.asarray(inputs["bout"], dtype=np.float32)
    r, E3, D = Wqkv.shape
    Bb, S, _ = x.shape

    nc = get_nc()
    in_maps = make_in_maps(x, Wqkv, bqkv, Wout, bout)
    res = run_bass_kernel_spmd(nc, in_maps, core_ids=list(range(len(in_maps))),
                               trace=trace, **kwargs)

    out = np.zeros((Bb, S, r * D), np.float32)
    for c in range(len(in_maps)):
        o, b = c // Bb, c % Bb
        out[b, o::r, o * D:(o + 1) * D] = \
            np.asarray(res.results[c]["outT"]).astype(np.float32).T
    return out, res


def kernel(x, Wqkv, bqkv, Wout, bout, num_heads):
    assert int(num_heads) == H0
    out, _ = run(dict(x=x, Wqkv=Wqkv, bqkv=bqkv, Wout=Wout, bout=bout))
    return out
